# revision 1
# baseline (speedup 1.0000x reference)
"""Causal attention block (QKV proj + RoPE + causal SDPA + out proj) on 8
Trainium2 NeuronCores.

Sharding: core c = 4*b + g handles batch b (of 2) and head group g (of 4,
4 heads each).  Each core computes q/k/v for its 4 heads from x[b] and the
matching Wqkv column slices, runs causal SDPA, and contracts its 512
output-feature rows of Wproj, producing a partial projT [2048, 2048].  The
host sums the 4 partials per batch (the "all-reduce") and transposes.

All matmuls run in float32r (fp32 data, 1 cycle/row on the PE when the
moving free dim >= 256; ~1.5e-4 relative error at K=256).

Device layouts (per core):
  xT    [C=2048, N=2048]  x[b] transposed (contraction dim C on partitions)
  wq/wk/wv [2048, 512]    Wqkv column slices for this head group
  wp    [512, 2048]       Wproj rows for this head group
  cosT  [128, 2048]       RoPE cos, head-dim on partitions
  sinT  [128, 2048]       RoPE sin, head-dim on partitions, first 64
                          partitions negated (folds rotate_half's sign)
  ones  [128, 128]        all-ones (rowsum via matmul)
  tri   [128, 128]        tri[j, i] = 1 if i >= j else 0 (causal diag mask)
  projT [2048, 2048] out  partial output, transposed

Inside: q^T/k^T computed per head as [hd=128, tok] (RoPE applied with
partition-half swap), v as [tok, hd].  Scores are computed transposed
(scT[j, i] = k_j . q_i) so softmax-exp tiles feed the attn@v matmul with no
transposes anywhere.  Softmax skips max-subtraction (|scores| <= ~8 here,
exp is safe in fp32); row sums come from an all-ones matmul and are divided
out after the attn@v accumulation.
"""

import sys

if "/opt/trn_rl_repo" not in sys.path:
    sys.path.insert(0, "/opt/trn_rl_repo")

from contextlib import ExitStack

import numpy as np

import concourse.bass as bass  # noqa: F401
import concourse.tile as tile
from concourse import bacc, bass_utils, mybir

F32 = mybir.dt.float32
F32R = mybir.dt.float32r
EXP = mybir.ActivationFunctionType.Exp

B, N, C = 2, 2048, 2048
H = 16  # total heads
HD = C // H  # 128
G = 4  # head groups (cores per batch)
HPG = H // G  # 4 heads per group
P = 128
PANEL = 512
NP = N // PANEL  # 4 token panels
KB = C // P  # 16 contraction blocks
SCALE = float(HD) ** -0.5
ROPE_BASE = 10000.0

_NC_CACHE = {}
DEBUG = False
REPS = 1
COMPUTE = True
PHASES = "ABC"
EXPBATCH = True


class _NoOpEngine:
    def __getattr__(self, name):
        return lambda *a, **k: None


def _emit(ctx, tc, t):
    nc = tc.nc
    const = ctx.enter_context(tc.tile_pool(name="const", bufs=1))
    xpool = ctx.enter_context(tc.tile_pool(name="x", bufs=2))
    qkv = ctx.enter_context(tc.tile_pool(name="qkv", bufs=1))
    epool = ctx.enter_context(tc.tile_pool(name="e", bufs=5))
    tmp = ctx.enter_context(tc.tile_pool(name="tmp", bufs=2))
    opool = ctx.enter_context(tc.tile_pool(name="o", bufs=1))
    pout = ctx.enter_context(tc.tile_pool(name="po", bufs=2))
    ps = ctx.enter_context(tc.tile_pool(name="ps", bufs=1, space="PSUM"))

    cosT = const.tile([P, N], F32)
    sinT = const.tile([P, N], F32)
    ones = const.tile([P, P], F32R)
    tri = const.tile([P, P], F32)
    nc.sync.dma_start(cosT, t["cosT"])
    nc.sync.dma_start(sinT, t["sinT"])
    nc.sync.dma_start(ones, t["ones"])
    nc.sync.dma_start(tri, t["tri"])

    outT = [
        opool.tile([P, N], F32R, tag=f"outT{h}", name=f"outT{h}") for h in range(HPG)
    ]

    xT3 = t["xT"].rearrange("(kb q) n -> q kb n", q=P)
    mm = nc.tensor.matmul

    if REPS == 1:
        _emit_once(tc, t, const, xpool, qkv, epool, tmp, opool, pout, ps,
                   cosT, sinT, ones, tri, outT, xT3, mm)
    else:
        with tc.For_i(0, REPS, 1):
            _emit_once(tc, t, const, xpool, qkv, epool, tmp, opool, pout, ps,
                       cosT, sinT, ones, tri, outT, xT3, mm)


def _emit_once(tc, t, const, xpool, qkv, epool, tmp, opool, pout, ps,
               cosT, sinT, ones, tri, outT, xT3, mm):
    nc = tc.nc
    vec = nc.vector if COMPUTE else _NoOpEngine()
    sca = nc.scalar if COMPUTE else _NoOpEngine()
    if not COMPUTE:
        mm = lambda *a, **k: None  # noqa: E731

    # wp (proj weights) shares the x pool slots (16KB each), loaded as halves
    wp_half = [None, None]

    def load_wp():
        wp3 = t["wp"].rearrange("(h p) o -> p h o", p=P)
        for i in range(2):
            wp_half[i] = xpool.tile([P, 2, N], F32R, tag="x", name=f"wp{i}")
            nc.sync.dma_start(wp_half[i], wp3[:, 2 * i : 2 * i + 2, :])

    def wp_block(h, obs):
        # lhsT tile [128, 128] for local head h, output block ob
        return wp_half[h // 2][:, h % 2, 128 * obs : 128 * (obs + 1)]

    def emit_proj_panel(p):
        sl = slice(PANEL * p, PANEL * (p + 1))
        for ob in range(KB):
            pj = ps.tile(
                [P, PANEL], F32, tag=f"V{2 + (ob % 2)}", name="pj"
            )
            for h in range(HPG):
                mm(
                    pj,
                    wp_block(h, ob),
                    outT[h][:, sl],
                    start=(h == 0),
                    stop=(h == HPG - 1),
                )
            if COMPUTE:
                o_t = pout.tile([P, PANEL], F32, tag="pout")
                if ob % 2 == 0:
                    sca.copy(o_t, pj)
                else:
                    vec.tensor_copy(o_t, pj)
                nc.sync.dma_start(t["projT"][128 * ob : 128 * (ob + 1), sl], o_t)
            else:
                nc.sync.dma_start(
                    t["projT"][128 * ob : 128 * (ob + 1), sl], cosT[:, 0:PANEL]
                )

    with tc.tile_pool(name="w", bufs=1) as wpool, tc.tile_pool(
        name="qkraw", bufs=2
    ) as rawpool:
        for sweep in range(2):
            # ---- phase A: QKV + RoPE for heads (2*sweep, 2*sweep+1) ----
            w_sb = {}
            for wname in ("wq", "wk", "wv"):
                w_sb[wname] = wpool.tile([P, KB, 256], F32R, tag=wname, name=wname)
                nc.sync.dma_start(
                    w_sb[wname],
                    t[wname].rearrange("(kb p) f -> p kb f", p=P)[
                        :, :, 256 * sweep : 256 * sweep + 256
                    ],
                )
            v_sb = qkv.tile([P, KB, 256], F32R, tag="v")
            qk = {}
            for hh in range(2):
                qk["q", hh] = qkv.tile([P, N], F32R, tag=f"q{hh}", name=f"q{hh}")
                qk["k", hh] = qkv.tile([P, N], F32R, tag=f"k{hh}", name=f"k{hh}")

            for p in range(NP):
                sl = slice(PANEL * p, PANEL * (p + 1))
                pq = [
                    ps.tile([P, PANEL], F32, tag=f"A{i}", name=f"pq{i}")
                    for i in range(2)
                ]
                pk = [
                    ps.tile([P, PANEL], F32, tag=f"A{i + 2}", name=f"pk{i}")
                    for i in range(2)
                ]
                pv = [
                    ps.tile([P, 256], F32, tag=f"V{tb}", name=f"pv{tb}")
                    for tb in range(4)
                ]
                for hb in range(2):
                    xt = xpool.tile([P, KB // 2, PANEL], F32R, tag="x")
                    nc.sync.dma_start(xt, xT3[:, 8 * hb : 8 * hb + 8, sl])
                    for kbl in range(KB // 2):
                        kb = 8 * hb + kbl
                        st, sp = kb == 0, kb == KB - 1
                        x_k = xt[:, kbl]
                        mm(pq[0], w_sb["wq"][:, kb, 0:128], x_k, start=st, stop=sp)
                        mm(pq[1], w_sb["wq"][:, kb, 128:256], x_k, start=st, stop=sp)
                        mm(pk[0], w_sb["wk"][:, kb, 0:128], x_k, start=st, stop=sp)
                        mm(pk[1], w_sb["wk"][:, kb, 128:256], x_k, start=st, stop=sp)
                        for tb in range(4):
                            mm(
                                pv[tb],
                                x_k[:, 128 * tb : 128 * (tb + 1)],
                                w_sb["wv"][:, kb],
                                start=st,
                                stop=sp,
                            )
                # Fast ACT copies free the q/k psum banks; RoPE runs on DVE
                # from SBUF off the critical path.
                # rope(q) = q*cos + swap64(q)*sin' (sin' pre-signed)
                for psrc, dst in (
                    (pq[0], qk["q", 0]),
                    (pq[1], qk["q", 1]),
                    (pk[0], qk["k", 0]),
                    (pk[1], qk["k", 1]),
                ):
                    raws = rawpool.tile([P, PANEL], F32, tag="raws")
                    rawsw = rawpool.tile([P, PANEL], F32, tag="rawsw")
                    sca.copy(raws, psrc)
                    sca.copy(rawsw[0:64], psrc[64:128])
                    sca.copy(rawsw[64:128], psrc[0:64])
                    t1 = tmp.tile([P, PANEL], F32, tag="rope1")
                    t2 = tmp.tile([P, PANEL], F32, tag="rope2")
                    vec.tensor_mul(t1, rawsw, sinT[:, sl])
                    vec.tensor_mul(t2, raws, cosT[:, sl])
                    vec.tensor_add(dst[:, sl], t2, t1)
                for tb in range(4):
                    sca.copy(v_sb[:, 4 * p + tb, :], pv[tb])

            # ---- phase B: causal SDPA, both heads; proj inlined on sweep 1
            if sweep == 1 and "C" in PHASES:
                load_wp()
            for p in range(NP if "B" in PHASES else 0):
                sl = slice(PANEL * p, PANEL * (p + 1))
                po = {}
                prs = {}
                e_tiles = {0: [], 1: []}
                for hh in range(2):
                    po[hh] = ps.tile(
                        [P, PANEL], F32, tag=f"V{hh}", name=f"po{hh}"
                    )
                    prs[hh] = ps.tile(
                        [P, PANEL], F32, tag=f"V{2 + hh}", name=f"prs{hh}"
                    )
                njb = 4 * p + 4

                def emit_av(hh, jj):
                    e_t, n0 = e_tiles[hh][jj]
                    st, sp = jj == 0, jj == njb - 1
                    mm(
                        po[hh][:, n0:],
                        v_sb[:, jj, 128 * hh : 128 * hh + 128],
                        e_t[:, n0:],
                        start=st,
                        stop=sp,
                    )
                    mm(prs[hh][:, n0:], ones, e_t[:, n0:], start=st, stop=sp)

                for jb in range(njb):
                    td = jb - 4 * p  # diagonal sub-block index if >= 0
                    n0 = 128 * td if td > 0 else 0
                    for hh in range(2):
                        if jb >= 3:
                            emit_av(hh, jb - 3)
                        sc1 = ps.tile(
                            [P, PANEL],
                            F32,
                            tag=f"A{(2 * jb + hh) % 4}",
                            name="sc1",
                        )
                        mm(
                            sc1[:, n0:],
                            qk["k", hh][:, 128 * jb : 128 * (jb + 1)],
                            qk["q", hh][:, PANEL * p + n0 : PANEL * (p + 1)],
                        )
                        e1 = epool.tile([P, PANEL], F32R, tag="e1")
                        sca.activation(e1[:, n0:], sc1[:, n0:], EXP, scale=SCALE)
                        if td >= 0:
                            dsl = slice(128 * td, 128 * (td + 1))
                            vec.tensor_mul(
                                e1[:, dsl], e1[:, dsl].bitcast(F32), tri
                            )
                        e_tiles[hh].append((e1, n0))
                for hh in range(2):
                    for jj in range(max(0, njb - 3), njb):
                        emit_av(hh, jj)
                    recip = tmp.tile([P, PANEL], F32, tag="rope1")
                    vec.reciprocal(recip, prs[hh])
                    vec.tensor_mul(
                        outT[2 * sweep + hh][:, sl], po[hh], recip
                    )
                if sweep == 1 and "C" in PHASES:
                    # proj for this panel: outT[0..3][:, sl] are all final now
                    emit_proj_panel(p)

    if DEBUG:
        for h in range(HPG):
            nc.sync.dma_start(t[f"dbg_o{h}"], outT[h].bitcast(F32))



def build_nc():
    key = (REPS, DEBUG, COMPUTE, PHASES, EXPBATCH)
    if key in _NC_CACHE:
        return _NC_CACHE[key]
    nc = bacc.Bacc("TRN2", target_bir_lowering=False, debug=False)
    t = {}
    t["xT"] = nc.dram_tensor("xT", [C, N], F32R, kind="ExternalInput").ap()
    t["wq"] = nc.dram_tensor("wq", [C, 512], F32R, kind="ExternalInput").ap()
    t["wk"] = nc.dram_tensor("wk", [C, 512], F32R, kind="ExternalInput").ap()
    t["wv"] = nc.dram_tensor("wv", [C, 512], F32R, kind="ExternalInput").ap()
    t["wp"] = nc.dram_tensor("wp", [512, N], F32R, kind="ExternalInput").ap()
    t["cosT"] = nc.dram_tensor("cosT", [P, N], F32, kind="ExternalInput").ap()
    t["sinT"] = nc.dram_tensor("sinT", [P, N], F32, kind="ExternalInput").ap()
    t["ones"] = nc.dram_tensor("ones", [P, P], F32R, kind="ExternalInput").ap()
    t["tri"] = nc.dram_tensor("tri", [P, P], F32, kind="ExternalInput").ap()
    t["projT"] = nc.dram_tensor("projT", [N, N], F32, kind="ExternalOutput").ap()
    if DEBUG:
        for h in range(HPG):
            t[f"dbg_q{h}"] = nc.dram_tensor(
                f"dbg_q{h}", [P, N], F32, kind="ExternalOutput"
            ).ap()
            t[f"dbg_k{h}"] = nc.dram_tensor(
                f"dbg_k{h}", [P, N], F32, kind="ExternalOutput"
            ).ap()
            t[f"dbg_o{h}"] = nc.dram_tensor(
                f"dbg_o{h}", [P, N], F32, kind="ExternalOutput"
            ).ap()
        for s in range(2):
            t[f"dbg_v{s}"] = nc.dram_tensor(
                f"dbg_v{s}", [N, 256], F32, kind="ExternalOutput"
            ).ap()
    with tile.TileContext(nc) as tc, ExitStack() as ctx:
        _emit(ctx, tc, t)
    nc.compile()
    _NC_CACHE[key] = nc
    return nc


def make_in_maps(x, position_ids, Wqkv, Wproj):
    x = np.asarray(x, dtype=np.float32)
    pos = np.asarray(position_ids, dtype=np.float64)
    Wqkv = np.asarray(Wqkv, dtype=np.float32)
    Wproj = np.asarray(Wproj, dtype=np.float32)

    inv_freq = 1.0 / (
        ROPE_BASE ** (np.arange(0, HD, 2, dtype=np.float32) / HD)
    )  # [64]
    ones = np.ones((P, P), dtype=np.float32)
    tri = (np.arange(P)[None, :] >= np.arange(P)[:, None]).astype(np.float32)

    in_maps = []
    for c in range(8):
        b, g = divmod(c, G)
        freqs = pos[b].astype(np.float32)[:, None] * inv_freq[None, :]  # [N, 64]
        emb = np.concatenate([freqs, freqs], axis=-1)  # [N, 128]
        cosT = np.ascontiguousarray(np.cos(emb).T)  # [128, N]
        sinT = np.sin(emb)
        sinT = np.ascontiguousarray(sinT.T)
        sinT[:64] = -sinT[:64]
        in_maps.append(
            {
                "xT": np.ascontiguousarray(x[b].T),
                "wq": np.ascontiguousarray(Wqkv[:, 512 * g : 512 * (g + 1)]),
                "wk": np.ascontiguousarray(
                    Wqkv[:, 2048 + 512 * g : 2048 + 512 * (g + 1)]
                ),
                "wv": np.ascontiguousarray(
                    Wqkv[:, 4096 + 512 * g : 4096 + 512 * (g + 1)]
                ),
                "wp": np.ascontiguousarray(Wproj[512 * g : 512 * (g + 1), :]),
                "cosT": cosT,
                "sinT": sinT,
                "ones": ones,
                "tri": tri,
            }
        )
    return in_maps


def kernel(x, position_ids, Wqkv, Wproj, _trace=False, _tmpdir=None):
    nc = build_nc()
    in_maps = make_in_maps(x, position_ids, Wqkv, Wproj)
    res = bass_utils.run_bass_kernel_spmd(
        nc, in_maps, core_ids=list(range(8)), trace=_trace, tmpdir=_tmpdir
    )
    out = np.empty((B, N, C), dtype=np.float32)
    for b in range(B):
        acc = res.results[4 * b]["projT"].copy()
        for g in range(1, G):
            acc += res.results[4 * b + g]["projT"]
        out[b] = acc.T
    kernel.last_exec_time_ns = res.exec_time_ns
    kernel.last_results = res
    return out



# revision 2
# speedup vs baseline: 1.1150x; 1.1150x over previous
"""Causal attention block (QKV proj + RoPE + causal SDPA + out proj) on 8
Trainium2 NeuronCores.

Sharding: core c = 4*b + g handles batch b (of 2) and head group g (of 4,
4 heads each).  Each core computes q/k/v for its 4 heads from x[b] and the
matching Wqkv column slices, runs causal SDPA, and contracts its 512
output-feature rows of Wproj, producing a partial projT [2048, 2048] in
bf16.  The host sums the 4 partials per batch in fp32 and transposes.

Precision/engine plan (rel tol 2e-2; measured ~5e-3):
  - QKV projection in fp32r (x, Wqkv fp32): full baseline precision, v and
    q/k raw values are exact-ish.
  - q/k stored bf16 after RoPE; score matmuls bf16 (absolute score noise
    ~6e-3, harmless through softmax).
  - attn weights e' = exp(s*scale - 3) written by ACT directly per key
    block.  The -3 bias keeps exp below fp8e4 max (240); it cancels in the
    softmax ratio.
  - panels 1-3 (queries 512+): e' in fp8, PAIR-PACKED [128, 2, 512]; attn@V
    and the all-ones rowsum are fp8 DoubleRow matmuls (K=256 keys/pass, 2x
    PE throughput); v stored fp8 [tok, 512feats].  Attention over >=512
    keys averages away the fp8 noise.
  - panel 0 (queries < 512, few keys -> no noise averaging): e' in fp32r,
    per-block fp32r attn@V + rowsum against an fp32 copy of the first 512
    v rows.
  - causal diagonal via a DVE min-mask (+-1e4) on PSUM scores before exp.
  - softmax normalization in fp32 (ACT evacuates PSUM, DVE recip + mul off
    the PE critical path); out-proj in bf16.

Per-core DMA: x fp32 16MB (streamed twice), Wqkv slices fp32 12MB, Wproj
slice bf16 2MB, cos/sin bf16 1MB, out partial bf16 8MB -> ~55MB vs 70MB
baseline, chunked for pipelining.
"""

import sys

if "/opt/trn_rl_repo" not in sys.path:
    sys.path.insert(0, "/opt/trn_rl_repo")

from contextlib import ExitStack

import ml_dtypes
import numpy as np

import concourse.bass as bass  # noqa: F401
import concourse.tile as tile
from concourse import bacc, bass_utils, mybir

F32 = mybir.dt.float32
F32R = mybir.dt.float32r
BF16 = mybir.dt.bfloat16
F8 = mybir.dt.float8e4
NBF16 = ml_dtypes.bfloat16
NF8 = ml_dtypes.float8_e4m3
EXP = mybir.ActivationFunctionType.Exp
DR = mybir.MatmulPerfMode.DoubleRow

B, N, C = 2, 2048, 2048
H = 16
HD = C // H  # 128
G = 4
HPG = H // G  # 4
P = 128
PANEL = 512
HPAN = 256  # phase-A half panel (SBUF-friendly x chunks)
NP = N // PANEL  # 4
KB = C // P  # 16
SCALE = float(HD) ** -0.5
EBIAS = -3.0
ROPE_BASE = 10000.0

_NC_CACHE = {}
DEBUG = False


def _emit(ctx, tc, t):
    nc = tc.nc
    mm = nc.tensor.matmul
    sca = nc.scalar
    vec = nc.vector

    const = ctx.enter_context(tc.tile_pool(name="const", bufs=1))
    xpool = ctx.enter_context(tc.tile_pool(name="x", bufs=2))
    wpool = ctx.enter_context(tc.tile_pool(name="w", bufs=1))
    qkpool = ctx.enter_context(tc.tile_pool(name="qk", bufs=1))
    vpool = ctx.enter_context(tc.tile_pool(name="v", bufs=1))
    e8pool = ctx.enter_context(tc.tile_pool(name="e8", bufs=3))
    efpool = ctx.enter_context(tc.tile_pool(name="ef", bufs=3))
    rawpool = ctx.enter_context(tc.tile_pool(name="raw", bufs=2))
    tmppool = ctx.enter_context(tc.tile_pool(name="tmp", bufs=2))
    normpool = ctx.enter_context(tc.tile_pool(name="nrm", bufs=2))
    opool = ctx.enter_context(tc.tile_pool(name="o", bufs=1))
    pout = ctx.enter_context(tc.tile_pool(name="po", bufs=2))
    ps = ctx.enter_context(tc.tile_pool(name="ps", bufs=1, space="PSUM"))

    # small consts first (cheap DMAs, needed early-ish)
    ones8 = const.tile([P, 2, P], F8)
    onesf = const.tile([P, P], F32R)
    mext = const.tile([P, 2 * P], F32)
    nc.sync.dma_start(ones8, t["ones8"])
    nc.sync.dma_start(onesf, t["onesf"])
    nc.sync.dma_start(mext, t["mext"])
    cosT = const.tile([P, N], BF16)
    sinT = const.tile([P, N], BF16)
    nc.gpsimd.dma_start(cosT, t["cosT"])
    nc.gpsimd.dma_start(sinT, t["sinT"])

    v_sb = vpool.tile([P, KB, PANEL], F8)  # all keys, 4 heads, fp8
    v4 = vpool.tile([P, 4, PANEL], F32R)  # first 512 keys, 4 heads, fp32
    outT = [opool.tile([P, N], BF16, name=f"outT{h}") for h in range(HPG)]

    xT3 = t["xT"].rearrange("(kb q) n -> q kb n", q=P)
    wv3 = t["wv"].rearrange("(kb p) f -> p kb f", p=P)
    wp3 = t["wp"].rearrange("(h p) o -> p h o", p=P)
    projT3 = t["projT"].rearrange("(ob q) n -> q ob n", q=P)

    wp_sb = None

    def load_wp():
        nonlocal wp_sb
        wp_sb = wpool.tile([P, HPG, N], BF16, tag="wv", name="wp")
        for i in range(HPG):
            nc.sync.dma_start(wp_sb[:, i : i + 1, :], wp3[:, i : i + 1, :])

    def emit_proj_panel(p):
        sl = slice(PANEL * p, PANEL * (p + 1))
        for ob2 in range(KB // 2):
            o_t = pout.tile([P, 2, PANEL], BF16, tag="pout")
            for c in range(2):
                ob = 2 * ob2 + c
                pj = ps.tile([P, PANEL], F32, tag=f"A{ob % 4}", name="pj")
                for h in range(HPG):
                    mm(
                        pj,
                        wp_sb[:, h, P * ob : P * (ob + 1)],
                        outT[h][:, sl],
                        start=(h == 0),
                        stop=(h == HPG - 1),
                    )
                if c == 0:
                    sca.copy(o_t[:, 0], pj)
                else:
                    vec.tensor_copy(o_t[:, 1], pj)
            nc.gpsimd.dma_start(projT3[:, 2 * ob2 : 2 * ob2 + 2, sl], o_t)

    for sweep in range(2):
        # ---- phase A: QKV + RoPE for heads (2*sweep, 2*sweep+1) ----
        # DMA order matters: first x half-panel, then first weight chunks,
        # so the first matmul can start within a few us.
        xt0 = xpool.tile([P, KB, HPAN], F32R, tag="x")
        nc.sync.dma_start(xt0[:, 0:4, :], xT3[:, 0:4, 0:HPAN])
        w_sb = {}
        wsrc = {}
        for wname in ("wq", "wk"):
            w_sb[wname] = wpool.tile([P, KB, 256], F32R, tag=wname, name=wname)
            wsrc[wname] = t[wname].rearrange("(kb p) f -> p kb f", p=P)[
                :, :, 256 * sweep : 256 * sweep + 256
            ]
            nc.sync.dma_start(w_sb[wname][:, 0:4, :], wsrc[wname][:, 0:4, :])
        nc.sync.dma_start(xt0[:, 4:8, :], xT3[:, 4:8, 0:HPAN])
        for wname in ("wq", "wk"):
            nc.sync.dma_start(w_sb[wname][:, 4:8, :], wsrc[wname][:, 4:8, :])
        nc.sync.dma_start(xt0[:, 8:16, :], xT3[:, 8:16, 0:HPAN])
        if sweep == 0:
            wv_sb = wpool.tile([P, KB, PANEL], F32R, tag="wv", name="wv")
            for ch in range(8):
                nc.sync.dma_start(
                    wv_sb[:, 2 * ch : 2 * ch + 2, :],
                    wv3[:, 2 * ch : 2 * ch + 2, :],
                )
        for wname in ("wq", "wk"):
            nc.sync.dma_start(w_sb[wname][:, 8:16, :], wsrc[wname][:, 8:16, :])

        qk = {}
        for hh in range(2):
            qk["q", hh] = qkpool.tile([P, N], BF16, tag=f"q{hh}", name=f"q{hh}")
            qk["k", hh] = qkpool.tile([P, N], BF16, tag=f"k{hh}", name=f"k{hh}")

        for hp in range(2 * NP):  # half-panels of 256 tokens
            sl = slice(HPAN * hp, HPAN * (hp + 1))
            bk = "A" if (sweep == 0 or hp % 2 == 0) else "V"
            pq = [
                ps.tile([P, HPAN], F32, tag=f"{bk}{i}", name=f"pq{i}")
                for i in range(2)
            ]
            pk = [
                ps.tile([P, HPAN], F32, tag=f"{bk}{i + 2}", name=f"pk{i}")
                for i in range(2)
            ]
            if sweep == 0:
                pv = [
                    ps.tile([P, PANEL], F32, tag=f"V{(2 * hp + i) % 4}", name=f"pv{i}")
                    for i in range(2)
                ]
            if hp == 0:
                xt = xt0
            else:
                xt = xpool.tile([P, KB, HPAN], F32R, tag="x")
                for half in range(2):
                    nc.sync.dma_start(
                        xt[:, 8 * half : 8 * half + 8, :],
                        xT3[:, 8 * half : 8 * half + 8, sl],
                    )
            for kb in range(KB):
                st, sp = kb == 0, kb == KB - 1
                x_k = xt[:, kb]
                mm(pq[0], w_sb["wq"][:, kb, 0:128], x_k, start=st, stop=sp)
                mm(pq[1], w_sb["wq"][:, kb, 128:256], x_k, start=st, stop=sp)
                mm(pk[0], w_sb["wk"][:, kb, 0:128], x_k, start=st, stop=sp)
                mm(pk[1], w_sb["wk"][:, kb, 128:256], x_k, start=st, stop=sp)
                if sweep == 0:
                    for tb in range(2):
                        mm(
                            pv[tb],
                            x_k[:, P * tb : P * (tb + 1)],
                            wv_sb[:, kb],
                            start=st,
                            stop=sp,
                        )
            # RoPE: rope(q) = q*cos + swap64(q)*sin' (sin' pre-signed)
            for psrc, dst in (
                (pq[0], qk["q", 0]),
                (pq[1], qk["q", 1]),
                (pk[0], qk["k", 0]),
                (pk[1], qk["k", 1]),
            ):
                rawsw = rawpool.tile([P, HPAN], BF16, tag="rawsw")
                sca.copy(rawsw[0:64], psrc[64:128])
                sca.copy(rawsw[64:128], psrc[0:64])
                t1 = tmppool.tile([P, HPAN], BF16, tag="rope1")
                t2 = tmppool.tile([P, HPAN], BF16, tag="rope2")
                vec.tensor_mul(t2, psrc, cosT[:, sl])
                vec.tensor_mul(t1, rawsw, sinT[:, sl])
                vec.tensor_add(dst[:, sl], t2, t1)
            if sweep == 0:
                for tb in range(2):
                    tbg = 2 * hp + tb
                    sca.copy(v_sb[:, tbg, :], pv[tb])
                    if tbg < 4:
                        sca.copy(v4[:, tbg, :], pv[tb])

        # ---- phase B: causal SDPA for the 2 heads; proj inlined sweep 1
        if sweep == 1:
            load_wp()
        for p in range(NP):
            sl = slice(PANEL * p, PANEL * (p + 1))
            po = {}
            prs = {}
            for hh in range(2):
                po[hh] = ps.tile([P, PANEL], F32, tag=f"V{hh}", name=f"po{hh}")
                prs[hh] = ps.tile([P, PANEL], F32, tag=f"V{2 + hh}", name=f"prs{hh}")

            def emit_sc(hh, jb, n0, mask_lo):
                """scores for key block jb over query cols [n0:], exp into
                the given e destination; mask_lo = start of masked region
                (None = no mask)."""
                sc = ps.tile([P, PANEL], F32, tag=f"A{jb % 4}", name="sc")
                mm(
                    sc[:, n0:],
                    qk["k", hh][:, P * jb : P * (jb + 1)],
                    qk["q", hh][:, PANEL * p + n0 : PANEL * (p + 1)],
                )
                td = jb - 4 * p
                if td >= 0:
                    lo = mask_lo if mask_lo is not None else P * td
                    width = P * (td + 1) - lo
                    vec.tensor_tensor(
                        sc[:, lo : P * (td + 1)],
                        sc[:, lo : P * (td + 1)],
                        mext[:, 2 * P - width :],
                        mybir.AluOpType.min,
                    )
                return sc

            if p == 0:
                # fp32r path: few keys per query -> no fp8 noise averaging
                e_tiles = {0: [], 1: []}

                def emit_av0(hh, jb):
                    e_t, n0 = e_tiles[hh][jb]
                    st, sp = jb == 0, jb == 3
                    h = 2 * sweep + hh
                    mm(
                        po[hh][:, n0:],
                        v4[:, jb, P * h : P * (h + 1)],
                        e_t[:, n0:],
                        start=st,
                        stop=sp,
                    )
                    mm(prs[hh][:, n0:], onesf, e_t[:, n0:], start=st, stop=sp)

                for jb in range(4):
                    n0 = P * jb
                    for hh in range(2):
                        if jb >= 2:
                            emit_av0(hh, jb - 2)
                        sc = emit_sc(hh, jb, n0, None)
                        e_t = efpool.tile([P, PANEL], F32R, tag=f"ef{hh}")
                        sca.activation(
                            e_t[:, n0:], sc[:, n0:], EXP, scale=SCALE, bias=EBIAS
                        )
                        e_tiles[hh].append((e_t, n0))
                for hh in range(2):
                    for jb in range(2, 4):
                        emit_av0(hh, jb)
            else:
                # fp8 DoubleRow path, two key blocks per matmul
                npair = 2 * p + 2
                e_tiles = {0: [], 1: []}

                def emit_av(hh, j):
                    e_t, pn0 = e_tiles[hh][j]
                    st, sp = j == 0, j == npair - 1
                    h = 2 * sweep + hh
                    mm(
                        po[hh][:, pn0:],
                        v_sb[:, 2 * j : 2 * j + 2, P * h : P * (h + 1)],
                        e_t[:, :, pn0:],
                        start=st,
                        stop=sp,
                        perf_mode=DR,
                    )
                    mm(
                        prs[hh][:, pn0:],
                        ones8,
                        e_t[:, :, pn0:],
                        start=st,
                        stop=sp,
                        perf_mode=DR,
                    )

                for j in range(npair):
                    pn0 = 256 if j == 2 * p + 1 else 0
                    for hh in range(2):
                        if j >= 2:
                            emit_av(hh, j - 2)
                        e_t = e8pool.tile([P, 2, PANEL], F8, tag=f"e{hh}")
                        for cc in range(2):
                            jb = 2 * j + cc
                            sc = emit_sc(
                                hh, jb, pn0, pn0 if cc == 1 else None
                            )
                            sca.activation(
                                e_t[:, cc, pn0:],
                                sc[:, pn0:],
                                EXP,
                                scale=SCALE,
                                bias=EBIAS,
                            )
                        e_tiles[hh].append((e_t, pn0))
                for hh in range(2):
                    for j in range(max(0, npair - 2), npair):
                        emit_av(hh, j)

            # evacuate PSUM fast (ACT), then normalize off-path (DVE, fp32)
            for hh in range(2):
                h = 2 * sweep + hh
                praw_t = normpool.tile([P, PANEL], F32, tag="praw")
                rs_t = normpool.tile([P, PANEL], F32, tag="rst")
                sca.copy(praw_t, po[hh])
                vec.tensor_copy(rs_t, prs[hh])
                rinv = normpool.tile([P, PANEL], F32, tag="rinv")
                vec.reciprocal(rinv, rs_t)
                vec.tensor_mul(outT[h][:, sl], praw_t, rinv)
            if sweep == 1 and p > 0:
                emit_proj_panel(p - 1)

    emit_proj_panel(NP - 1)

    if DEBUG:
        for h in range(HPG):
            nc.sync.dma_start(t[f"dbg_o{h}"], outT[h])


def build_nc():
    key = (DEBUG,)
    if key in _NC_CACHE:
        return _NC_CACHE[key]
    nc = bacc.Bacc("TRN2", target_bir_lowering=False, debug=False)
    # register the exp bias as a const AP (same pattern as Bacc.__init__)
    _bt = nc.alloc_sbuf_tensor(f"const-float32-{EBIAS}", [128, 1], F32)
    nc.gpsimd.memset(_bt.ap(), EBIAS)
    nc.const_aps.aps[(F32, EBIAS)] = _bt.ap()
    nc.all_engine_barrier()
    t = {}
    t["xT"] = nc.dram_tensor("xT", [C, N], F32R, kind="ExternalInput").ap()
    t["wq"] = nc.dram_tensor("wq", [C, 512], F32R, kind="ExternalInput").ap()
    t["wk"] = nc.dram_tensor("wk", [C, 512], F32R, kind="ExternalInput").ap()
    t["wv"] = nc.dram_tensor("wv", [C, 512], F32R, kind="ExternalInput").ap()
    t["wp"] = nc.dram_tensor("wp", [512, N], BF16, kind="ExternalInput").ap()
    t["cosT"] = nc.dram_tensor("cosT", [P, N], BF16, kind="ExternalInput").ap()
    t["sinT"] = nc.dram_tensor("sinT", [P, N], BF16, kind="ExternalInput").ap()
    t["ones8"] = nc.dram_tensor("ones8", [P, 2, P], F8, kind="ExternalInput").ap()
    t["onesf"] = nc.dram_tensor("onesf", [P, P], F32R, kind="ExternalInput").ap()
    t["mext"] = nc.dram_tensor("mext", [P, 2 * P], F32, kind="ExternalInput").ap()
    t["projT"] = nc.dram_tensor("projT", [N, N], BF16, kind="ExternalOutput").ap()
    if DEBUG:
        for h in range(HPG):
            t[f"dbg_o{h}"] = nc.dram_tensor(
                f"dbg_o{h}", [P, N], BF16, kind="ExternalOutput"
            ).ap()
    with tile.TileContext(nc) as tc, ExitStack() as ctx:
        _emit(ctx, tc, t)
    nc.compile()
    _NC_CACHE[key] = nc
    return nc


def make_in_maps(x, position_ids, Wqkv, Wproj):
    x = np.asarray(x, dtype=np.float32)
    pos = np.asarray(position_ids, dtype=np.float64)
    Wqkv = np.asarray(Wqkv, dtype=np.float32)
    Wproj = np.asarray(Wproj, dtype=np.float32)

    inv_freq = 1.0 / (ROPE_BASE ** (np.arange(0, HD, 2, dtype=np.float32) / HD))
    ones8 = np.ones((P, 2, P), dtype=NF8)
    onesf = np.ones((P, P), dtype=np.float32)
    # mext[:, 0:128] = all -1e4 (strip); mext[:, 128:256] = tri (+1e4 keep)
    tri = np.where(
        np.arange(P)[None, :] >= np.arange(P)[:, None], 1e4, -1e4
    ).astype(np.float32)
    mext = np.concatenate([np.full((P, P), -1e4, dtype=np.float32), tri], axis=1)

    in_maps = []
    for c in range(8):
        b, g = divmod(c, G)
        freqs = pos[b].astype(np.float32)[:, None] * inv_freq[None, :]
        emb = np.concatenate([freqs, freqs], axis=-1)  # [N, 128]
        cosT = np.ascontiguousarray(np.cos(emb).T)
        sinT = np.sin(emb)
        sinT = np.ascontiguousarray(sinT.T)
        sinT[:64] = -sinT[:64]
        in_maps.append(
            {
                "xT": np.ascontiguousarray(x[b].T),
                "wq": np.ascontiguousarray(Wqkv[:, 512 * g : 512 * (g + 1)]),
                "wk": np.ascontiguousarray(
                    Wqkv[:, 2048 + 512 * g : 2048 + 512 * (g + 1)]
                ),
                "wv": np.ascontiguousarray(
                    Wqkv[:, 4096 + 512 * g : 4096 + 512 * (g + 1)]
                ),
                "wp": np.ascontiguousarray(
                    Wproj[512 * g : 512 * (g + 1), :]
                ).astype(NBF16),
                "cosT": cosT.astype(NBF16),
                "sinT": sinT.astype(NBF16),
                "ones8": ones8,
                "onesf": onesf,
                "mext": mext,
            }
        )
    return in_maps


def kernel(x, position_ids, Wqkv, Wproj, _trace=False, _tmpdir=None):
    nc = build_nc()
    in_maps = make_in_maps(x, position_ids, Wqkv, Wproj)
    res = bass_utils.run_bass_kernel_spmd(
        nc, in_maps, core_ids=list(range(8)), trace=_trace, tmpdir=_tmpdir
    )
    out = np.empty((B, N, C), dtype=np.float32)
    for b in range(B):
        acc = res.results[4 * b]["projT"].astype(np.float32)
        for g in range(1, G):
            acc += res.results[4 * b + g]["projT"].astype(np.float32)
        out[b] = acc.T
    kernel.last_exec_time_ns = res.exec_time_ns
    kernel.last_results = res
    return out


# revision 3
# speedup vs baseline: 1.1455x; 1.0273x over previous
"""Causal attention block (QKV proj + RoPE + causal SDPA + out proj) on 8
Trainium2 NeuronCores — pipelined v3.

Sharding: core c = 4*b + g handles batch b (of 2) and head group g (of 4
heads).  Host sums the 4 bf16 projT partials per batch in fp32.

v3 structure: phase B (attention) is ACT(exp)-bound, so it runs with a
4-PSUM-bank footprint ({A0,A1} score rotation, {V0,V1} po/prs, one head at
a time) while OTHER matmul work runs on the remaining 4 banks
({A2,A3,V2,V3}), interleaved at ~1us emission granularity via generators:

  A0 (QKV+RoPE heads 0-1, v for all heads)  -> [B0  x  A1(QKV heads 2-3)]
  -> [B1 x out-proj(panels 0-2)] -> out-proj(panel 3)

Precision (tol 2e-2, measured ~4e-3): QKV fp32r; q/k bf16 after RoPE
(scores bf16); e' = exp(s*scale-3) -> fp8 pair-packed [128,2,512] for
panels 1-3 with attn@V + ones-rowsum as fp8 DoubleRow matmuls (2x PE);
panel 0 (few keys -> no noise averaging) in bf16; the -3 bias keeps exp
under fp8 max and cancels in the softmax ratio; causal diagonal via DVE
min-mask (+-1e4) on PSUM before exp; normalization fp32 with
reciprocal_approx_fast off the PE path; out-proj bf16.
"""

import sys

if "/opt/trn_rl_repo" not in sys.path:
    sys.path.insert(0, "/opt/trn_rl_repo")

from contextlib import ExitStack

import ml_dtypes
import numpy as np

import concourse.bass as bass  # noqa: F401
import concourse.tile as tile
from concourse import bacc, bass_utils, mybir

F32 = mybir.dt.float32
F32R = mybir.dt.float32r
BF16 = mybir.dt.bfloat16
F8 = mybir.dt.float8e4
NBF16 = ml_dtypes.bfloat16
NF8 = ml_dtypes.float8_e4m3
EXP = mybir.ActivationFunctionType.Exp
DR = mybir.MatmulPerfMode.DoubleRow

B, N, C = 2, 2048, 2048
H = 16
HD = C // H  # 128
G = 4
HPG = H // G  # 4
P = 128
PANEL = 512
HPAN = 256
NP = N // PANEL  # 4
KB = C // P  # 16
SCALE = float(HD) ** -0.5
EBIAS = -3.0
ROPE_BASE = 10000.0

_NC_CACHE = {}
DEBUG = False


def _emit(ctx, tc, t):
    nc = tc.nc
    mm = nc.tensor.matmul
    sca = nc.scalar
    vec = nc.vector

    const = ctx.enter_context(tc.tile_pool(name="const", bufs=1))
    xpool = ctx.enter_context(tc.tile_pool(name="x", bufs=2))
    wpool = ctx.enter_context(tc.tile_pool(name="w", bufs=1))
    qkpool = ctx.enter_context(tc.tile_pool(name="qk", bufs=1))
    vpool = ctx.enter_context(tc.tile_pool(name="v", bufs=1))
    e8pool = ctx.enter_context(tc.tile_pool(name="e8", bufs=3))
    efpool = ctx.enter_context(tc.tile_pool(name="ef", bufs=3))
    rawpool = ctx.enter_context(tc.tile_pool(name="raw", bufs=2))
    tmppool = ctx.enter_context(tc.tile_pool(name="tmp", bufs=2))
    normpool = ctx.enter_context(tc.tile_pool(name="nrm", bufs=2))
    opool = ctx.enter_context(tc.tile_pool(name="o", bufs=1))
    pout = ctx.enter_context(tc.tile_pool(name="po", bufs=2))
    ps = ctx.enter_context(tc.tile_pool(name="ps", bufs=1, space="PSUM"))

    ones8 = const.tile([P, 2, P], F8)
    onesb = const.tile([P, P], BF16)
    mext = const.tile([P, 2 * P], F32)
    nc.sync.dma_start(ones8, t["ones8"])
    nc.sync.dma_start(onesb, t["onesb"])
    nc.sync.dma_start(mext, t["mext"])
    cosT = const.tile([P, N], BF16)
    sinT = const.tile([P, N], BF16)
    nc.gpsimd.dma_start(cosT, t["cosT"])
    nc.gpsimd.dma_start(sinT, t["sinT"])

    v_sb = vpool.tile([P, KB, PANEL], F8)  # all keys, 4 heads, fp8
    v4 = vpool.tile([P, 4, PANEL], BF16)  # first 512 keys, 4 heads, bf16
    outT = [opool.tile([P, N], BF16, name=f"outT{h}") for h in range(HPG)]

    xT3 = t["xT"].rearrange("(kb q) n -> q kb n", q=P)
    wv3 = t["wv"].rearrange("(kb p) f -> p kb f", p=P)
    wp3 = t["wp"].rearrange("(h p) o -> p h o", p=P)
    projT3 = t["projT"].rearrange("(ob q) n -> q ob n", q=P)

    qk = {}
    for s in range(2):
        for hh in range(2):
            qk["q", hh, s] = qkpool.tile(
                [P, N], BF16, tag=f"q{hh}s{s}", name=f"q{hh}s{s}"
            )
            qk["k", hh, s] = qkpool.tile(
                [P, N], BF16, tag=f"k{hh}s{s}", name=f"k{hh}s{s}"
            )

    wp_sb = [None]

    # ---------------- phase A generator (QKV + RoPE) ----------------
    def gen_A(sweep):
        # bank sets: sweep 0 owns everything; sweep 1 (interleaved with
        # B0) uses only {A2, A3, V2, V3}
        if sweep == 0:
            qtag = ["A0", "A1"]
            ktag = ["A2", "A3"]
        else:
            qtag = ["A2", "A3"]
            ktag = ["V2", "V3"]

        xt0 = xpool.tile([P, KB, HPAN], F32R, tag="x")
        nc.sync.dma_start(xt0[:, 0:4, :], xT3[:, 0:4, 0:HPAN])
        w_sb = {}
        wsrc = {}
        for wname in ("wq", "wk"):
            w_sb[wname] = wpool.tile([P, KB, 256], F32R, tag=wname, name=wname)
            wsrc[wname] = t[wname].rearrange("(kb p) f -> p kb f", p=P)[
                :, :, 256 * sweep : 256 * sweep + 256
            ]
            nc.sync.dma_start(w_sb[wname][:, 0:4, :], wsrc[wname][:, 0:4, :])
        nc.sync.dma_start(xt0[:, 4:8, :], xT3[:, 4:8, 0:HPAN])
        for wname in ("wq", "wk"):
            nc.sync.dma_start(w_sb[wname][:, 4:8, :], wsrc[wname][:, 4:8, :])
        nc.sync.dma_start(xt0[:, 8:16, :], xT3[:, 8:16, 0:HPAN])
        if sweep == 0:
            wv_sb = wpool.tile([P, KB, PANEL], F32R, tag="wv", name="wv")
            for ch in range(8):
                nc.gpsimd.dma_start(
                    wv_sb[:, 2 * ch : 2 * ch + 2, :],
                    wv3[:, 2 * ch : 2 * ch + 2, :],
                )
        else:
            # prefetch wp for the proj phase (reuses the wv slot)
            wp_sb[0] = wpool.tile([P, HPG, N], BF16, tag="wv", name="wp")
            for i in range(HPG):
                nc.gpsimd.dma_start(wp_sb[0][:, i : i + 1, :], wp3[:, i : i + 1, :])
        for wname in ("wq", "wk"):
            nc.sync.dma_start(w_sb[wname][:, 8:16, :], wsrc[wname][:, 8:16, :])
        yield

        for hp in range(2 * NP):
            sl = slice(HPAN * hp, HPAN * (hp + 1))
            pq = [
                ps.tile([P, HPAN], F32, tag=qtag[i], name=f"pq{i}")
                for i in range(2)
            ]
            pk = [
                ps.tile([P, HPAN], F32, tag=ktag[i], name=f"pk{i}")
                for i in range(2)
            ]
            if sweep == 0:
                pv = [
                    ps.tile([P, PANEL], F32, tag=f"V{(2 * hp + i) % 4}", name=f"pv{i}")
                    for i in range(2)
                ]
            if hp == 0:
                xt = xt0
            else:
                xt = xpool.tile([P, KB, HPAN], F32R, tag="x")
                for half in range(2):
                    nc.sync.dma_start(
                        xt[:, 8 * half : 8 * half + 8, :],
                        xT3[:, 8 * half : 8 * half + 8, sl],
                    )
            for kb4 in range(4):
                for kbl in range(4):
                    kb = 4 * kb4 + kbl
                    st, sp = kb == 0, kb == KB - 1
                    x_k = xt[:, kb]
                    mm(pq[0], w_sb["wq"][:, kb, 0:128], x_k, start=st, stop=sp)
                    mm(pq[1], w_sb["wq"][:, kb, 128:256], x_k, start=st, stop=sp)
                    mm(pk[0], w_sb["wk"][:, kb, 0:128], x_k, start=st, stop=sp)
                    mm(pk[1], w_sb["wk"][:, kb, 128:256], x_k, start=st, stop=sp)
                    if sweep == 0:
                        for tb in range(2):
                            mm(
                                pv[tb],
                                x_k[:, P * tb : P * (tb + 1)],
                                wv_sb[:, kb],
                                start=st,
                                stop=sp,
                            )
                yield
            for psrc, dst in (
                (pq[0], qk["q", 0, sweep]),
                (pq[1], qk["q", 1, sweep]),
                (pk[0], qk["k", 0, sweep]),
                (pk[1], qk["k", 1, sweep]),
            ):
                rawsw = rawpool.tile([P, HPAN], BF16, tag="rawsw")
                sca.copy(rawsw[0:64], psrc[64:128])
                vec.tensor_copy(rawsw[64:128], psrc[0:64])
                t1 = tmppool.tile([P, HPAN], BF16, tag="rope1")
                t2 = tmppool.tile([P, HPAN], BF16, tag="rope2")
                vec.tensor_mul(t2, psrc, cosT[:, sl])
                vec.tensor_mul(t1, rawsw, sinT[:, sl])
                vec.tensor_add(dst[:, sl], t2, t1)
            if sweep == 0:
                for tb in range(2):
                    tbg = 2 * hp + tb
                    sca.copy(v_sb[:, tbg, :], pv[tb])
                    if tbg < 4:
                        sca.copy(v4[:, tbg, :], pv[tb])
            yield

    # ---------------- phase B generator (causal SDPA) ----------------
    def gen_B(sweep):
        for p in range(NP):
            sl = slice(PANEL * p, PANEL * (p + 1))
            nrm = []
            for hh in range(2):
                h = 2 * sweep + hh
                po = ps.tile([P, PANEL], F32, tag="V0", name="po")
                prs = ps.tile([P, PANEL], F32, tag="V1", name="prs")

                def emit_sc(jb, n0, mask_lo):
                    sc = ps.tile([P, PANEL], F32, tag=f"A{jb % 2}", name="sc")
                    mm(
                        sc[:, n0:],
                        qk["k", hh, sweep][:, P * jb : P * (jb + 1)],
                        qk["q", hh, sweep][:, PANEL * p + n0 : PANEL * (p + 1)],
                    )
                    td = jb - 4 * p
                    if td >= 0:
                        lo = mask_lo if mask_lo is not None else P * td
                        width = P * (td + 1) - lo
                        vec.tensor_tensor(
                            sc[:, lo : P * (td + 1)],
                            sc[:, lo : P * (td + 1)],
                            mext[:, 2 * P - width :],
                            mybir.AluOpType.min,
                        )
                    return sc

                if p == 0:
                    e_tiles = []

                    def emit_av0(jb):
                        e_t, n0 = e_tiles[jb]
                        st, sp = jb == 0, jb == 3
                        mm(
                            po[:, n0:],
                            v4[:, jb, P * h : P * (h + 1)],
                            e_t[:, n0:],
                            start=st,
                            stop=sp,
                        )
                        mm(prs[:, n0:], onesb, e_t[:, n0:], start=st, stop=sp)

                    for jb in range(4):
                        n0 = P * jb
                        if jb >= 2:
                            emit_av0(jb - 2)
                        sc = emit_sc(jb, n0, None)
                        e_t = efpool.tile([P, PANEL], BF16, tag="ef")
                        sca.activation(
                            e_t[:, n0:], sc[:, n0:], EXP, scale=SCALE, bias=EBIAS
                        )
                        e_tiles.append((e_t, n0))
                        yield
                    for jb in range(2, 4):
                        emit_av0(jb)
                else:
                    npair = 2 * p + 2
                    e_tiles = []

                    def emit_av(j):
                        e_t, pn0 = e_tiles[j]
                        st, sp = j == 0, j == npair - 1
                        mm(
                            po[:, pn0:],
                            v_sb[:, 2 * j : 2 * j + 2, P * h : P * (h + 1)],
                            e_t[:, :, pn0:],
                            start=st,
                            stop=sp,
                            perf_mode=DR,
                        )
                        mm(
                            prs[:, pn0:],
                            ones8,
                            e_t[:, :, pn0:],
                            start=st,
                            stop=sp,
                            perf_mode=DR,
                        )

                    for j in range(npair):
                        pn0 = 256 if j == 2 * p + 1 else 0
                        if j >= 2:
                            emit_av(j - 2)
                        e_t = e8pool.tile([P, 2, PANEL], F8, tag="e8")
                        for cc in range(2):
                            jb = 2 * j + cc
                            sc = emit_sc(jb, pn0, pn0 if cc == 1 else None)
                            sca.activation(
                                e_t[:, cc, pn0:],
                                sc[:, pn0:],
                                EXP,
                                scale=SCALE,
                                bias=EBIAS,
                            )
                        e_tiles.append((e_t, pn0))
                        yield
                    for j in range(max(0, npair - 2), npair):
                        emit_av(j)

                # evacuate this head's po/prs so the banks free up
                praw_t = normpool.tile([P, PANEL], F32, tag=f"praw{hh}")
                rs_t = normpool.tile([P, PANEL], F32, tag=f"rst{hh}")
                sca.copy(praw_t, po)
                vec.tensor_copy(rs_t, prs)
                nrm.append((praw_t, rs_t))
                yield
            for hh in range(2):
                h = 2 * sweep + hh
                praw_t, rs_t = nrm[hh]
                rinv = normpool.tile([P, PANEL], F32, tag=f"rinv{hh}")
                vec.reciprocal_approx_fast(rinv, rs_t)
                vec.tensor_mul(outT[h][:, sl], praw_t, rinv)
            yield

    # ---------------- out-proj generator ----------------
    def gen_proj():
        for p in range(NP):
            sl = slice(PANEL * p, PANEL * (p + 1))
            for ob2 in range(KB // 2):
                o_t = pout.tile([P, 2, PANEL], BF16, tag="pout")
                for c in range(2):
                    ob = 2 * ob2 + c
                    bank = ("A2", "A3", "V2", "V3")[ob % 4]
                    pj = ps.tile([P, PANEL], F32, tag=bank, name="pj")
                    for hx in range(HPG):
                        mm(
                            pj,
                            wp_sb[0][:, hx, P * ob : P * (ob + 1)],
                            outT[hx][:, sl],
                            start=(hx == 0),
                            stop=(hx == HPG - 1),
                        )
                    if c == 0:
                        sca.copy(o_t[:, 0], pj)
                    else:
                        vec.tensor_copy(o_t[:, 1], pj)
                    yield
                nc.gpsimd.dma_start(projT3[:, 2 * ob2 : 2 * ob2 + 2, sl], o_t)

    def drive(primary, secondary, ratio):
        """Interleave: per primary yield, pull `ratio` secondary yields."""
        for _ in primary:
            for _ in range(ratio):
                if secondary is not None and next(secondary, "END") == "END":
                    secondary = None
        while secondary is not None and next(secondary, "END") != "END":
            pass

    # A0 alone (owns all banks)
    for _ in gen_A(0):
        pass
    # B0 interleaved with A1
    drive(gen_B(0), gen_A(1), 2)
    # B1 interleaved with proj(panels 0..2); panel p's proj only becomes
    # emittable after its norm, which drive() handles via emission order:
    # proj is the secondary and trails B1 by construction of gen order.
    drive(gen_B(1), gen_proj(), 2)

    if DEBUG:
        for h in range(HPG):
            nc.sync.dma_start(t[f"dbg_o{h}"], outT[h])


def build_nc():
    key = (DEBUG,)
    if key in _NC_CACHE:
        return _NC_CACHE[key]
    nc = bacc.Bacc("TRN2", target_bir_lowering=False, debug=False)
    _bt = nc.alloc_sbuf_tensor(f"const-float32-{EBIAS}", [128, 1], F32)
    nc.gpsimd.memset(_bt.ap(), EBIAS)
    nc.const_aps.aps[(F32, EBIAS)] = _bt.ap()
    nc.all_engine_barrier()
    t = {}
    t["xT"] = nc.dram_tensor("xT", [C, N], F32R, kind="ExternalInput").ap()
    t["wq"] = nc.dram_tensor("wq", [C, 512], F32R, kind="ExternalInput").ap()
    t["wk"] = nc.dram_tensor("wk", [C, 512], F32R, kind="ExternalInput").ap()
    t["wv"] = nc.dram_tensor("wv", [C, 512], F32R, kind="ExternalInput").ap()
    t["wp"] = nc.dram_tensor("wp", [512, N], BF16, kind="ExternalInput").ap()
    t["cosT"] = nc.dram_tensor("cosT", [P, N], BF16, kind="ExternalInput").ap()
    t["sinT"] = nc.dram_tensor("sinT", [P, N], BF16, kind="ExternalInput").ap()
    t["ones8"] = nc.dram_tensor("ones8", [P, 2, P], F8, kind="ExternalInput").ap()
    t["onesb"] = nc.dram_tensor("onesb", [P, P], BF16, kind="ExternalInput").ap()
    t["mext"] = nc.dram_tensor("mext", [P, 2 * P], F32, kind="ExternalInput").ap()
    t["projT"] = nc.dram_tensor("projT", [N, N], BF16, kind="ExternalOutput").ap()
    if DEBUG:
        for h in range(HPG):
            t[f"dbg_o{h}"] = nc.dram_tensor(
                f"dbg_o{h}", [P, N], BF16, kind="ExternalOutput"
            ).ap()
    with tile.TileContext(nc) as tc, ExitStack() as ctx:
        _emit(ctx, tc, t)
    nc.compile()
    _NC_CACHE[key] = nc
    return nc


def make_in_maps(x, position_ids, Wqkv, Wproj):
    x = np.asarray(x, dtype=np.float32)
    pos = np.asarray(position_ids, dtype=np.float64)
    Wqkv = np.asarray(Wqkv, dtype=np.float32)
    Wproj = np.asarray(Wproj, dtype=np.float32)

    inv_freq = 1.0 / (ROPE_BASE ** (np.arange(0, HD, 2, dtype=np.float32) / HD))
    ones8 = np.ones((P, 2, P), dtype=NF8)
    onesb = np.ones((P, P), dtype=NBF16)
    tri = np.where(
        np.arange(P)[None, :] >= np.arange(P)[:, None], 1e4, -1e4
    ).astype(np.float32)
    mext = np.concatenate([np.full((P, P), -1e4, dtype=np.float32), tri], axis=1)

    in_maps = []
    for c in range(8):
        b, g = divmod(c, G)
        freqs = pos[b].astype(np.float32)[:, None] * inv_freq[None, :]
        emb = np.concatenate([freqs, freqs], axis=-1)
        cosT = np.ascontiguousarray(np.cos(emb).T)
        sinT = np.sin(emb)
        sinT = np.ascontiguousarray(sinT.T)
        sinT[:64] = -sinT[:64]
        in_maps.append(
            {
                "xT": np.ascontiguousarray(x[b].T),
                "wq": np.ascontiguousarray(Wqkv[:, 512 * g : 512 * (g + 1)]),
                "wk": np.ascontiguousarray(
                    Wqkv[:, 2048 + 512 * g : 2048 + 512 * (g + 1)]
                ),
                "wv": np.ascontiguousarray(
                    Wqkv[:, 4096 + 512 * g : 4096 + 512 * (g + 1)]
                ),
                "wp": np.ascontiguousarray(
                    Wproj[512 * g : 512 * (g + 1), :]
                ).astype(NBF16),
                "cosT": cosT.astype(NBF16),
                "sinT": sinT.astype(NBF16),
                "ones8": ones8,
                "onesb": onesb,
                "mext": mext,
            }
        )
    return in_maps


def kernel(x, position_ids, Wqkv, Wproj, _trace=False, _tmpdir=None):
    nc = build_nc()
    in_maps = make_in_maps(x, position_ids, Wqkv, Wproj)
    res = bass_utils.run_bass_kernel_spmd(
        nc, in_maps, core_ids=list(range(8)), trace=_trace, tmpdir=_tmpdir
    )
    out = np.empty((B, N, C), dtype=np.float32)
    for b in range(B):
        acc = res.results[4 * b]["projT"].astype(np.float32)
        for g in range(1, G):
            acc += res.results[4 * b + g]["projT"].astype(np.float32)
        out[b] = acc.T
    kernel.last_exec_time_ns = res.exec_time_ns
    kernel.last_results = res
    return out


# revision 4
# speedup vs baseline: 1.2021x; 1.0494x over previous
"""Causal attention block (QKV proj + RoPE + causal SDPA + out proj) on 8
Trainium2 NeuronCores — pipelined v3.

Sharding: core c = 4*b + g handles batch b (of 2) and head group g (of 4
heads).  Host sums the 4 bf16 projT partials per batch in fp32.

v3 structure: phase B (attention) is ACT(exp)-bound, so it runs with a
4-PSUM-bank footprint ({A0,A1} score rotation, {V0,V1} po/prs, one head at
a time) while OTHER matmul work runs on the remaining 4 banks
({A2,A3,V2,V3}), interleaved at ~1us emission granularity via generators:

  A0 (QKV+RoPE heads 0-1, v for all heads)  -> [B0  x  A1(QKV heads 2-3)]
  -> [B1 x out-proj(panels 0-2)] -> out-proj(panel 3)

Precision (tol 2e-2, measured ~4e-3): QKV fp32r; q/k bf16 after RoPE
(scores bf16); e' = exp(s*scale-3) -> fp8 pair-packed [128,2,512] for
panels 1-3 with attn@V + ones-rowsum as fp8 DoubleRow matmuls (2x PE);
panel 0 (few keys -> no noise averaging) in bf16; the -3 bias keeps exp
under fp8 max and cancels in the softmax ratio; causal diagonal via DVE
min-mask (+-1e4) on PSUM before exp; normalization fp32 with
reciprocal_approx_fast off the PE path; out-proj bf16.
"""

import sys

if "/opt/trn_rl_repo" not in sys.path:
    sys.path.insert(0, "/opt/trn_rl_repo")

from contextlib import ExitStack

import ml_dtypes
import numpy as np

import concourse.bass as bass  # noqa: F401
import concourse.tile as tile
from concourse import bacc, bass_utils, mybir

F32 = mybir.dt.float32
F32R = mybir.dt.float32r
BF16 = mybir.dt.bfloat16
F8 = mybir.dt.float8e4
NBF16 = ml_dtypes.bfloat16
NF8 = ml_dtypes.float8_e4m3
EXP = mybir.ActivationFunctionType.Exp
DR = mybir.MatmulPerfMode.DoubleRow

B, N, C = 2, 2048, 2048
H = 16
HD = C // H  # 128
G = 4
HPG = H // G  # 4
P = 128
PANEL = 512
HPAN = 256
NP = N // PANEL  # 4
KB = C // P  # 16
SCALE = float(HD) ** -0.5
EBIAS = -3.0
ROPE_BASE = 10000.0

_NC_CACHE = {}
DEBUG = False


def _emit(ctx, tc, t):
    nc = tc.nc
    mm = nc.tensor.matmul
    sca = nc.scalar
    vec = nc.vector

    const = ctx.enter_context(tc.tile_pool(name="const", bufs=1))
    xpool = ctx.enter_context(tc.tile_pool(name="x", bufs=2))
    wpool = ctx.enter_context(tc.tile_pool(name="w", bufs=1))
    qkpool = ctx.enter_context(tc.tile_pool(name="qk", bufs=1))
    vpool = ctx.enter_context(tc.tile_pool(name="v", bufs=1))
    e8pool = ctx.enter_context(tc.tile_pool(name="e8", bufs=3))
    efpool = ctx.enter_context(tc.tile_pool(name="ef", bufs=3))
    rawpool = ctx.enter_context(tc.tile_pool(name="raw", bufs=2))
    tmppool = ctx.enter_context(tc.tile_pool(name="tmp", bufs=2))
    normpool = ctx.enter_context(tc.tile_pool(name="nrm", bufs=2))
    opool = ctx.enter_context(tc.tile_pool(name="o", bufs=1))
    pout = ctx.enter_context(tc.tile_pool(name="po", bufs=2))
    ps = ctx.enter_context(tc.tile_pool(name="ps", bufs=1, space="PSUM"))

    ones8 = const.tile([P, 2, P], F8)
    onesb = const.tile([P, P], BF16)
    mext = const.tile([P, 2 * P], F32)
    nc.sync.dma_start(ones8, t["ones8"])
    nc.sync.dma_start(onesb, t["onesb"])
    nc.sync.dma_start(mext, t["mext"])
    cosT = const.tile([P, N], BF16)
    sinT = const.tile([P, N], BF16)
    nc.gpsimd.dma_start(cosT, t["cosT"])
    nc.gpsimd.dma_start(sinT, t["sinT"])

    v_sb = vpool.tile([P, KB, PANEL], F8)  # all keys, 4 heads, fp8
    v4 = vpool.tile([P, 4, PANEL], BF16)  # first 512 keys, 4 heads, bf16
    outT = [opool.tile([P, N], BF16, name=f"outT{h}") for h in range(HPG)]

    xT3 = t["xT"].rearrange("(kb q) n -> q kb n", q=P)
    wv3 = t["wv"].rearrange("(kb p) f -> p kb f", p=P)
    wp3 = t["wp"].rearrange("(h p) o -> p h o", p=P)
    projT3 = t["projT"].rearrange("(ob q) n -> q ob n", q=P)

    qk = {}
    for s in range(2):
        for hh in range(2):
            qk["q", hh, s] = qkpool.tile(
                [P, N], BF16, tag=f"q{hh}s{s}", name=f"q{hh}s{s}"
            )
            qk["k", hh, s] = qkpool.tile(
                [P, N], BF16, tag=f"k{hh}s{s}", name=f"k{hh}s{s}"
            )

    wp_sb = [None]

    # ---------------- phase A generator (QKV + RoPE) ----------------
    def gen_A(sweep):
        # bank sets: sweep 0 owns everything; sweep 1 (interleaved with
        # B0) uses only {A2, A3, V2, V3}
        if sweep == 0:
            qtag = ["A0", "A1"]
            ktag = ["A2", "A3"]
        else:
            qtag = ["A2", "A3"]
            ktag = ["V2", "V3"]

        xt0 = xpool.tile([P, KB, HPAN], F32R, tag="x")
        nc.sync.dma_start(xt0[:, 0:2, :], xT3[:, 0:2, 0:HPAN])
        w_sb = {}
        wsrc = {}
        for wname in ("wq", "wk"):
            w_sb[wname] = wpool.tile([P, KB, 256], F32R, tag=wname, name=wname)
            wsrc[wname] = t[wname].rearrange("(kb p) f -> p kb f", p=P)[
                :, :, 256 * sweep : 256 * sweep + 256
            ]
            nc.sync.dma_start(w_sb[wname][:, 0:2, :], wsrc[wname][:, 0:2, :])
        nc.sync.dma_start(xt0[:, 2:8, :], xT3[:, 2:8, 0:HPAN])
        for wname in ("wq", "wk"):
            nc.sync.dma_start(w_sb[wname][:, 2:8, :], wsrc[wname][:, 2:8, :])
        nc.sync.dma_start(xt0[:, 8:16, :], xT3[:, 8:16, 0:HPAN])
        if sweep == 0:
            wv_sb = wpool.tile([P, KB, PANEL], F32R, tag="wv", name="wv")
            for ch in range(8):
                nc.gpsimd.dma_start(
                    wv_sb[:, 2 * ch : 2 * ch + 2, :],
                    wv3[:, 2 * ch : 2 * ch + 2, :],
                )
        else:
            # prefetch wp for the proj phase (reuses the wv slot)
            wp_sb[0] = wpool.tile([P, HPG, N], BF16, tag="wv", name="wp")
            for i in range(HPG):
                nc.gpsimd.dma_start(wp_sb[0][:, i : i + 1, :], wp3[:, i : i + 1, :])
        for wname in ("wq", "wk"):
            nc.sync.dma_start(w_sb[wname][:, 8:16, :], wsrc[wname][:, 8:16, :])
        yield 0

        for hp in range(2 * NP):
            sl = slice(HPAN * hp, HPAN * (hp + 1))
            pq = [
                ps.tile([P, HPAN], F32, tag=qtag[i], name=f"pq{i}")
                for i in range(2)
            ]
            pk = [
                ps.tile([P, HPAN], F32, tag=ktag[i], name=f"pk{i}")
                for i in range(2)
            ]
            if sweep == 0:
                pv = [
                    ps.tile([P, PANEL], F32, tag=f"V{(2 * hp + i) % 4}", name=f"pv{i}")
                    for i in range(2)
                ]
            if hp == 0:
                xt = xt0
            else:
                xt = xpool.tile([P, KB, HPAN], F32R, tag="x")
                for half in range(2):
                    nc.sync.dma_start(
                        xt[:, 8 * half : 8 * half + 8, :],
                        xT3[:, 8 * half : 8 * half + 8, sl],
                    )
            for kb4 in range(4):
                for kbl in range(4):
                    kb = 4 * kb4 + kbl
                    st, sp = kb == 0, kb == KB - 1
                    x_k = xt[:, kb]
                    mm(pq[0], w_sb["wq"][:, kb, 0:128], x_k, start=st, stop=sp)
                    mm(pq[1], w_sb["wq"][:, kb, 128:256], x_k, start=st, stop=sp)
                    mm(pk[0], w_sb["wk"][:, kb, 0:128], x_k, start=st, stop=sp)
                    mm(pk[1], w_sb["wk"][:, kb, 128:256], x_k, start=st, stop=sp)
                    if sweep == 0:
                        for tb in range(2):
                            mm(
                                pv[tb],
                                x_k[:, P * tb : P * (tb + 1)],
                                wv_sb[:, kb],
                                start=st,
                                stop=sp,
                            )
                yield 8192 if sweep == 0 else 4096
            for psrc, dst in (
                (pq[0], qk["q", 0, sweep]),
                (pq[1], qk["q", 1, sweep]),
                (pk[0], qk["k", 0, sweep]),
                (pk[1], qk["k", 1, sweep]),
            ):
                rawsw = rawpool.tile([P, HPAN], BF16, tag="rawsw")
                sca.copy(rawsw[0:64], psrc[64:128])
                vec.tensor_copy(rawsw[64:128], psrc[0:64])
                t1 = tmppool.tile([P, HPAN], BF16, tag="rope1")
                t2 = tmppool.tile([P, HPAN], BF16, tag="rope2")
                vec.tensor_mul(t2, psrc, cosT[:, sl])
                vec.tensor_mul(t1, rawsw, sinT[:, sl])
                vec.tensor_add(dst[:, sl], t2, t1)
            if sweep == 0:
                for tb in range(2):
                    tbg = 2 * hp + tb
                    sca.copy(v_sb[:, tbg, :], pv[tb])
                    if tbg < 4:
                        sca.copy(v4[:, tbg, :], pv[tb])
            yield 200

    # ---------------- phase B generator (causal SDPA) ----------------
    def gen_B(sweep):
        for p in range(NP):
            sl = slice(PANEL * p, PANEL * (p + 1))
            nrm = []
            for hh in range(2):
                h = 2 * sweep + hh
                po = ps.tile([P, PANEL], F32, tag="V0", name="po")
                prs = ps.tile([P, PANEL], F32, tag="V1", name="prs")

                def emit_sc(jb, n0, mask_lo):
                    sc = ps.tile([P, PANEL], F32, tag=f"A{jb % 2}", name="sc")
                    mm(
                        sc[:, n0:],
                        qk["k", hh, sweep][:, P * jb : P * (jb + 1)],
                        qk["q", hh, sweep][:, PANEL * p + n0 : PANEL * (p + 1)],
                    )
                    td = jb - 4 * p
                    if td >= 0:
                        lo = mask_lo if mask_lo is not None else P * td
                        width = P * (td + 1) - lo
                        vec.tensor_tensor(
                            sc[:, lo : P * (td + 1)],
                            sc[:, lo : P * (td + 1)],
                            mext[:, 2 * P - width :],
                            mybir.AluOpType.min,
                        )
                    return sc

                if p == 0:
                    e_tiles = []

                    def emit_av0(jb):
                        e_t, n0 = e_tiles[jb]
                        st, sp = jb == 0, jb == 3
                        mm(
                            po[:, n0:],
                            v4[:, jb, P * h : P * (h + 1)],
                            e_t[:, n0:],
                            start=st,
                            stop=sp,
                        )
                        mm(prs[:, n0:], onesb, e_t[:, n0:], start=st, stop=sp)

                    for jb in range(4):
                        n0 = P * jb
                        if jb >= 2:
                            emit_av0(jb - 2)
                        sc = emit_sc(jb, n0, None)
                        e_t = efpool.tile([P, PANEL], BF16, tag="ef")
                        sca.activation(
                            e_t[:, n0:], sc[:, n0:], EXP, scale=SCALE, bias=EBIAS
                        )
                        e_tiles.append((e_t, n0))
                        yield (512 - n0) * 5 // 2
                    for jb in range(2, 4):
                        emit_av0(jb)
                else:
                    npair = 2 * p + 2
                    e_tiles = []

                    def emit_av(j):
                        e_t, pn0 = e_tiles[j]
                        st, sp = j == 0, j == npair - 1
                        mm(
                            po[:, pn0:],
                            v_sb[:, 2 * j : 2 * j + 2, P * h : P * (h + 1)],
                            e_t[:, :, pn0:],
                            start=st,
                            stop=sp,
                            perf_mode=DR,
                        )
                        mm(
                            prs[:, pn0:],
                            ones8,
                            e_t[:, :, pn0:],
                            start=st,
                            stop=sp,
                            perf_mode=DR,
                        )

                    for j in range(npair):
                        pn0 = 256 if j == 2 * p + 1 else 0
                        if j >= 2:
                            emit_av(j - 2)
                        e_t = e8pool.tile([P, 2, PANEL], F8, tag="e8")
                        for cc in range(2):
                            jb = 2 * j + cc
                            sc = emit_sc(jb, pn0, pn0 if cc == 1 else None)
                            sca.activation(
                                e_t[:, cc, pn0:],
                                sc[:, pn0:],
                                EXP,
                                scale=SCALE,
                                bias=EBIAS,
                            )
                        e_tiles.append((e_t, pn0))
                        yield 3 * (512 - pn0)
                    for j in range(max(0, npair - 2), npair):
                        emit_av(j)

                # evacuate this head's po/prs so the banks free up
                praw_t = normpool.tile([P, PANEL], F32, tag=f"praw{hh}")
                rs_t = normpool.tile([P, PANEL], F32, tag=f"rst{hh}")
                sca.copy(praw_t, po)
                vec.tensor_copy(rs_t, prs)
                nrm.append((praw_t, rs_t))
                yield 100
            for hh in range(2):
                h = 2 * sweep + hh
                praw_t, rs_t = nrm[hh]
                rinv = normpool.tile([P, PANEL], F32, tag=f"rinv{hh}")
                vec.reciprocal_approx_fast(rinv, rs_t)
                vec.tensor_mul(outT[h][:, sl], praw_t, rinv)
            yield 100

    # ---------------- out-proj generator ----------------
    def gen_proj():
        for p in range(NP):
            sl = slice(PANEL * p, PANEL * (p + 1))
            for ob2 in range(KB // 2):
                o_t = pout.tile([P, 2, PANEL], BF16, tag="pout")
                for c in range(2):
                    ob = 2 * ob2 + c
                    bank = ("A2", "A3", "V2", "V3")[ob % 4]
                    pj = ps.tile([P, PANEL], F32, tag=bank, name="pj")
                    for hx in range(HPG):
                        mm(
                            pj,
                            wp_sb[0][:, hx, P * ob : P * (ob + 1)],
                            outT[hx][:, sl],
                            start=(hx == 0),
                            stop=(hx == HPG - 1),
                        )
                    if c == 0:
                        sca.copy(o_t[:, 0], pj)
                    else:
                        vec.tensor_copy(o_t[:, 1], pj)
                    yield 2048
                nc.gpsimd.dma_start(projT3[:, 2 * ob2 : 2 * ob2 + 2, sl], o_t)

    def drive(primary, secondary, ratio):
        """Interleave by emitted-PE-cost: keep secondary's emitted cost at
        ~ratio x primary's, so both generators drain together."""
        pc, sc_acc = 1.0, 0.0
        for c in primary:
            pc += c
            while secondary is not None and sc_acc < pc * ratio:
                nx = next(secondary, None)
                if nx is None:
                    secondary = None
                else:
                    sc_acc += nx
        while secondary is not None:
            if next(secondary, None) is None:
                secondary = None

    # A0 alone (owns all banks)
    for _ in gen_A(0):
        pass
    # B0 interleaved with A1
    drive(gen_B(0), gen_A(1), 2.2)
    # B1 interleaved with proj(panels 0..2); panel p's proj only becomes
    # emittable after its norm, which drive() handles via emission order:
    # proj is the secondary and trails B1 by construction of gen order.
    drive(gen_B(1), gen_proj(), 2.2)

    if DEBUG:
        for h in range(HPG):
            nc.sync.dma_start(t[f"dbg_o{h}"], outT[h])


def build_nc():
    key = (DEBUG,)
    if key in _NC_CACHE:
        return _NC_CACHE[key]
    nc = bacc.Bacc("TRN2", target_bir_lowering=False, debug=False)
    _bt = nc.alloc_sbuf_tensor(f"const-float32-{EBIAS}", [128, 1], F32)
    nc.gpsimd.memset(_bt.ap(), EBIAS)
    nc.const_aps.aps[(F32, EBIAS)] = _bt.ap()
    nc.all_engine_barrier()
    t = {}
    t["xT"] = nc.dram_tensor("xT", [C, N], F32R, kind="ExternalInput").ap()
    t["wq"] = nc.dram_tensor("wq", [C, 512], F32R, kind="ExternalInput").ap()
    t["wk"] = nc.dram_tensor("wk", [C, 512], F32R, kind="ExternalInput").ap()
    t["wv"] = nc.dram_tensor("wv", [C, 512], F32R, kind="ExternalInput").ap()
    t["wp"] = nc.dram_tensor("wp", [512, N], BF16, kind="ExternalInput").ap()
    t["cosT"] = nc.dram_tensor("cosT", [P, N], BF16, kind="ExternalInput").ap()
    t["sinT"] = nc.dram_tensor("sinT", [P, N], BF16, kind="ExternalInput").ap()
    t["ones8"] = nc.dram_tensor("ones8", [P, 2, P], F8, kind="ExternalInput").ap()
    t["onesb"] = nc.dram_tensor("onesb", [P, P], BF16, kind="ExternalInput").ap()
    t["mext"] = nc.dram_tensor("mext", [P, 2 * P], F32, kind="ExternalInput").ap()
    t["projT"] = nc.dram_tensor("projT", [N, N], BF16, kind="ExternalOutput").ap()
    if DEBUG:
        for h in range(HPG):
            t[f"dbg_o{h}"] = nc.dram_tensor(
                f"dbg_o{h}", [P, N], BF16, kind="ExternalOutput"
            ).ap()
    with tile.TileContext(nc) as tc, ExitStack() as ctx:
        _emit(ctx, tc, t)
    nc.compile()
    _NC_CACHE[key] = nc
    return nc


def make_in_maps(x, position_ids, Wqkv, Wproj):
    x = np.asarray(x, dtype=np.float32)
    pos = np.asarray(position_ids, dtype=np.float64)
    Wqkv = np.asarray(Wqkv, dtype=np.float32)
    Wproj = np.asarray(Wproj, dtype=np.float32)

    inv_freq = 1.0 / (ROPE_BASE ** (np.arange(0, HD, 2, dtype=np.float32) / HD))
    ones8 = np.ones((P, 2, P), dtype=NF8)
    onesb = np.ones((P, P), dtype=NBF16)
    tri = np.where(
        np.arange(P)[None, :] >= np.arange(P)[:, None], 1e4, -1e4
    ).astype(np.float32)
    mext = np.concatenate([np.full((P, P), -1e4, dtype=np.float32), tri], axis=1)

    in_maps = []
    for c in range(8):
        b, g = divmod(c, G)
        freqs = pos[b].astype(np.float32)[:, None] * inv_freq[None, :]
        emb = np.concatenate([freqs, freqs], axis=-1)
        cosT = np.ascontiguousarray(np.cos(emb).T)
        sinT = np.sin(emb)
        sinT = np.ascontiguousarray(sinT.T)
        sinT[:64] = -sinT[:64]
        in_maps.append(
            {
                "xT": np.ascontiguousarray(x[b].T),
                "wq": np.ascontiguousarray(Wqkv[:, 512 * g : 512 * (g + 1)]),
                "wk": np.ascontiguousarray(
                    Wqkv[:, 2048 + 512 * g : 2048 + 512 * (g + 1)]
                ),
                "wv": np.ascontiguousarray(
                    Wqkv[:, 4096 + 512 * g : 4096 + 512 * (g + 1)]
                ),
                "wp": np.ascontiguousarray(
                    Wproj[512 * g : 512 * (g + 1), :]
                ).astype(NBF16),
                "cosT": cosT.astype(NBF16),
                "sinT": sinT.astype(NBF16),
                "ones8": ones8,
                "onesb": onesb,
                "mext": mext,
            }
        )
    return in_maps


def kernel(x, position_ids, Wqkv, Wproj, _trace=False, _tmpdir=None):
    nc = build_nc()
    in_maps = make_in_maps(x, position_ids, Wqkv, Wproj)
    res = bass_utils.run_bass_kernel_spmd(
        nc, in_maps, core_ids=list(range(8)), trace=_trace, tmpdir=_tmpdir
    )
    out = np.empty((B, N, C), dtype=np.float32)
    for b in range(B):
        acc = res.results[4 * b]["projT"].astype(np.float32)
        for g in range(1, G):
            acc += res.results[4 * b + g]["projT"].astype(np.float32)
        out[b] = acc.T
    kernel.last_exec_time_ns = res.exec_time_ns
    kernel.last_results = res
    return out


# revision 5
# speedup vs baseline: 1.2088x; 1.0056x over previous
"""Causal attention block (QKV proj + RoPE + causal SDPA + out proj) on 8
Trainium2 NeuronCores — pipelined v3.

Sharding: core c = 4*b + g handles batch b (of 2) and head group g (of 4
heads).  Host sums the 4 bf16 projT partials per batch in fp32.

v3 structure: phase B (attention) is ACT(exp)-bound, so it runs with a
4-PSUM-bank footprint ({A0,A1} score rotation, {V0,V1} po/prs, one head at
a time) while OTHER matmul work runs on the remaining 4 banks
({A2,A3,V2,V3}), interleaved at ~1us emission granularity via generators:

  A0 (QKV+RoPE heads 0-1, v for all heads)  -> [B0  x  A1(QKV heads 2-3)]
  -> [B1 x out-proj(panels 0-2)] -> out-proj(panel 3)

Precision (tol 2e-2, measured ~4e-3): QKV fp32r; q/k bf16 after RoPE
(scores bf16); e' = exp(s*scale-3) -> fp8 pair-packed [128,2,512] for
panels 1-3 with attn@V + ones-rowsum as fp8 DoubleRow matmuls (2x PE);
panel 0 (few keys -> no noise averaging) in bf16; the -3 bias keeps exp
under fp8 max and cancels in the softmax ratio; causal diagonal via DVE
min-mask (+-1e4) on PSUM before exp; normalization fp32 with
reciprocal_approx_fast off the PE path; out-proj bf16.
"""

import sys

if "/opt/trn_rl_repo" not in sys.path:
    sys.path.insert(0, "/opt/trn_rl_repo")

from contextlib import ExitStack

import ml_dtypes
import numpy as np

import concourse.bass as bass  # noqa: F401
import concourse.tile as tile
from concourse import bacc, bass_utils, mybir

F32 = mybir.dt.float32
F32R = mybir.dt.float32r
BF16 = mybir.dt.bfloat16
F8 = mybir.dt.float8e4
NBF16 = ml_dtypes.bfloat16
NF8 = ml_dtypes.float8_e4m3
EXP = mybir.ActivationFunctionType.Exp
DR = mybir.MatmulPerfMode.DoubleRow

B, N, C = 2, 2048, 2048
H = 16
HD = C // H  # 128
G = 4
HPG = H // G  # 4
P = 128
PANEL = 512
HPAN = 256
NP = N // PANEL  # 4
KB = C // P  # 16
SCALE = float(HD) ** -0.5
EBIAS = -3.0
ROPE_BASE = 10000.0

_NC_CACHE = {}
DEBUG = False


def _emit(ctx, tc, t):
    nc = tc.nc
    mm = nc.tensor.matmul
    sca = nc.scalar
    vec = nc.vector

    const = ctx.enter_context(tc.tile_pool(name="const", bufs=1))
    xpool = ctx.enter_context(tc.tile_pool(name="x", bufs=2))
    wpool = ctx.enter_context(tc.tile_pool(name="w", bufs=1))
    qkpool = ctx.enter_context(tc.tile_pool(name="qk", bufs=1))
    vpool = ctx.enter_context(tc.tile_pool(name="v", bufs=1))
    e8pool = ctx.enter_context(tc.tile_pool(name="e8", bufs=3))
    efpool = ctx.enter_context(tc.tile_pool(name="ef", bufs=3))
    rawpool = ctx.enter_context(tc.tile_pool(name="raw", bufs=2))
    tmppool = ctx.enter_context(tc.tile_pool(name="tmp", bufs=2))
    normpool = ctx.enter_context(tc.tile_pool(name="nrm", bufs=2))
    opool = ctx.enter_context(tc.tile_pool(name="o", bufs=1))
    pout = ctx.enter_context(tc.tile_pool(name="po", bufs=2))
    ps = ctx.enter_context(tc.tile_pool(name="ps", bufs=1, space="PSUM"))

    ones8 = const.tile([P, 2, P], F8)
    onesb = const.tile([P, P], BF16)
    mext = const.tile([P, 2 * P], F32)
    nc.gpsimd.dma_start(ones8, t["ones8"])
    nc.gpsimd.dma_start(onesb, t["onesb"])
    nc.gpsimd.dma_start(mext, t["mext"])
    cosT = const.tile([P, N], BF16)
    sinT = const.tile([P, N], BF16)
    nc.gpsimd.dma_start(cosT, t["cosT"])
    nc.gpsimd.dma_start(sinT, t["sinT"])

    v_sb = vpool.tile([P, KB, PANEL], F8)  # all keys, 4 heads, fp8
    v4 = vpool.tile([P, 4, PANEL], BF16)  # first 512 keys, 4 heads, bf16
    outT = [opool.tile([P, N], BF16, name=f"outT{h}") for h in range(HPG)]

    xT3 = t["xT"].rearrange("(kb q) n -> q kb n", q=P)
    wv3 = t["wv"].rearrange("(kb p) f -> p kb f", p=P)
    wp3 = t["wp"].rearrange("(h p) o -> p h o", p=P)
    projT3 = t["projT"].rearrange("(ob q) n -> q ob n", q=P)

    qk = {}
    for s in range(2):
        for hh in range(2):
            qk["q", hh, s] = qkpool.tile(
                [P, N], BF16, tag=f"q{hh}s{s}", name=f"q{hh}s{s}"
            )
            qk["k", hh, s] = qkpool.tile(
                [P, N], BF16, tag=f"k{hh}s{s}", name=f"k{hh}s{s}"
            )

    wp_sb = [None]

    # ---------------- phase A generator (QKV + RoPE) ----------------
    def gen_A(sweep):
        # bank sets: sweep 0 owns everything; sweep 1 (interleaved with
        # B0) uses only {A2, A3, V2, V3}
        if sweep == 0:
            qtag = ["A0", "A1"]
            ktag = ["A2", "A3"]
        else:
            qtag = ["A2", "A3"]
            ktag = ["V2", "V3"]

        xt0 = xpool.tile([P, KB, HPAN], BF16, tag="x")
        nc.sync.dma_start(xt0[:, 0:2, :], xT3[:, 0:2, 0:HPAN])
        w_sb = {}
        wsrc = {}
        for wname in ("wq", "wk"):
            w_sb[wname] = wpool.tile([P, KB, 256], BF16, tag=wname, name=wname)
            wsrc[wname] = t[wname].rearrange("(kb p) f -> p kb f", p=P)[
                :, :, 256 * sweep : 256 * sweep + 256
            ]
            nc.sync.dma_start(w_sb[wname][:, 0:2, :], wsrc[wname][:, 0:2, :])
        nc.sync.dma_start(xt0[:, 2:8, :], xT3[:, 2:8, 0:HPAN])
        for wname in ("wq", "wk"):
            nc.sync.dma_start(w_sb[wname][:, 2:8, :], wsrc[wname][:, 2:8, :])
        nc.sync.dma_start(xt0[:, 8:16, :], xT3[:, 8:16, 0:HPAN])
        if sweep == 0:
            wv_sb = wpool.tile([P, KB, PANEL], BF16, tag="wv", name="wv")
            for ch in range(8):
                nc.gpsimd.dma_start(
                    wv_sb[:, 2 * ch : 2 * ch + 2, :],
                    wv3[:, 2 * ch : 2 * ch + 2, :],
                )
        else:
            # prefetch wp for the proj phase (reuses the wv slot)
            wp_sb[0] = wpool.tile([P, HPG, N], BF16, tag="wv", name="wp")
            for i in range(HPG):
                nc.gpsimd.dma_start(wp_sb[0][:, i : i + 1, :], wp3[:, i : i + 1, :])
        for wname in ("wq", "wk"):
            nc.sync.dma_start(w_sb[wname][:, 8:16, :], wsrc[wname][:, 8:16, :])
        yield 0

        for hp in range(2 * NP):
            sl = slice(HPAN * hp, HPAN * (hp + 1))
            pq = [
                ps.tile([P, HPAN], F32, tag=qtag[i], name=f"pq{i}")
                for i in range(2)
            ]
            pk = [
                ps.tile([P, HPAN], F32, tag=ktag[i], name=f"pk{i}")
                for i in range(2)
            ]
            if sweep == 0:
                pv = [
                    ps.tile([P, PANEL], F32, tag=f"V{(2 * hp + i) % 4}", name=f"pv{i}")
                    for i in range(2)
                ]
            if hp == 0:
                xt = xt0
            else:
                xt = xpool.tile([P, KB, HPAN], BF16, tag="x")
                for half in range(2):
                    nc.sync.dma_start(
                        xt[:, 8 * half : 8 * half + 8, :],
                        xT3[:, 8 * half : 8 * half + 8, sl],
                    )
            for kb4 in range(4):
                for kbl in range(4):
                    kb = 4 * kb4 + kbl
                    st, sp = kb == 0, kb == KB - 1
                    x_k = xt[:, kb]
                    mm(pq[0], w_sb["wq"][:, kb, 0:128], x_k, start=st, stop=sp)
                    mm(pq[1], w_sb["wq"][:, kb, 128:256], x_k, start=st, stop=sp)
                    mm(pk[0], w_sb["wk"][:, kb, 0:128], x_k, start=st, stop=sp)
                    mm(pk[1], w_sb["wk"][:, kb, 128:256], x_k, start=st, stop=sp)
                    if sweep == 0:
                        for tb in range(2):
                            mm(
                                pv[tb],
                                x_k[:, P * tb : P * (tb + 1)],
                                wv_sb[:, kb],
                                start=st,
                                stop=sp,
                            )
                yield 8192 if sweep == 0 else 4096
            for psrc, dst in (
                (pq[0], qk["q", 0, sweep]),
                (pq[1], qk["q", 1, sweep]),
                (pk[0], qk["k", 0, sweep]),
                (pk[1], qk["k", 1, sweep]),
            ):
                rawsw = rawpool.tile([P, HPAN], BF16, tag="rawsw")
                sca.copy(rawsw[0:64], psrc[64:128])
                vec.tensor_copy(rawsw[64:128], psrc[0:64])
                t1 = tmppool.tile([P, HPAN], BF16, tag="rope1")
                t2 = tmppool.tile([P, HPAN], BF16, tag="rope2")
                vec.tensor_mul(t2, psrc, cosT[:, sl])
                vec.tensor_mul(t1, rawsw, sinT[:, sl])
                vec.tensor_add(dst[:, sl], t2, t1)
            if sweep == 0:
                for tb in range(2):
                    tbg = 2 * hp + tb
                    sca.copy(v_sb[:, tbg, :], pv[tb])
                    if tbg < 4:
                        sca.copy(v4[:, tbg, :], pv[tb])
            yield 200

    # ---------------- phase B generator (causal SDPA) ----------------
    def gen_B(sweep):
        for p in range(NP):
            sl = slice(PANEL * p, PANEL * (p + 1))
            nrm = []
            for hh in range(2):
                h = 2 * sweep + hh
                po = ps.tile([P, PANEL], F32, tag="V0", name="po")
                prs = ps.tile([P, PANEL], F32, tag="V1", name="prs")

                def emit_sc(jb, n0, mask_lo):
                    sc = ps.tile([P, PANEL], F32, tag=f"A{jb % 2}", name="sc")
                    mm(
                        sc[:, n0:],
                        qk["k", hh, sweep][:, P * jb : P * (jb + 1)],
                        qk["q", hh, sweep][:, PANEL * p + n0 : PANEL * (p + 1)],
                    )
                    td = jb - 4 * p
                    if td >= 0:
                        lo = mask_lo if mask_lo is not None else P * td
                        width = P * (td + 1) - lo
                        vec.tensor_tensor(
                            sc[:, lo : P * (td + 1)],
                            sc[:, lo : P * (td + 1)],
                            mext[:, 2 * P - width :],
                            mybir.AluOpType.min,
                        )
                    return sc

                if p == 0:
                    e_tiles = []

                    def emit_av0(jb):
                        e_t, n0 = e_tiles[jb]
                        st, sp = jb == 0, jb == 3
                        mm(
                            po[:, n0:],
                            v4[:, jb, P * h : P * (h + 1)],
                            e_t[:, n0:],
                            start=st,
                            stop=sp,
                        )
                        mm(prs[:, n0:], onesb, e_t[:, n0:], start=st, stop=sp)

                    for jb in range(4):
                        n0 = P * jb
                        if jb >= 2:
                            emit_av0(jb - 2)
                        sc = emit_sc(jb, n0, None)
                        e_t = efpool.tile([P, PANEL], BF16, tag="ef")
                        sca.activation(
                            e_t[:, n0:], sc[:, n0:], EXP, scale=SCALE, bias=EBIAS
                        )
                        e_tiles.append((e_t, n0))
                        yield (512 - n0) * 5 // 2
                    for jb in range(2, 4):
                        emit_av0(jb)
                else:
                    npair = 2 * p + 2
                    e_tiles = []

                    def emit_av(j):
                        e_t, pn0 = e_tiles[j]
                        st, sp = j == 0, j == npair - 1
                        mm(
                            po[:, pn0:],
                            v_sb[:, 2 * j : 2 * j + 2, P * h : P * (h + 1)],
                            e_t[:, :, pn0:],
                            start=st,
                            stop=sp,
                            perf_mode=DR,
                        )
                        mm(
                            prs[:, pn0:],
                            ones8,
                            e_t[:, :, pn0:],
                            start=st,
                            stop=sp,
                            perf_mode=DR,
                        )

                    for j in range(npair):
                        pn0 = 256 if j == 2 * p + 1 else 0
                        if j >= 2:
                            emit_av(j - 2)
                        e_t = e8pool.tile([P, 2, PANEL], F8, tag="e8")
                        for cc in range(2):
                            jb = 2 * j + cc
                            sc = emit_sc(jb, pn0, pn0 if cc == 1 else None)
                            sca.activation(
                                e_t[:, cc, pn0:],
                                sc[:, pn0:],
                                EXP,
                                scale=SCALE,
                                bias=EBIAS,
                            )
                        e_tiles.append((e_t, pn0))
                        yield 3 * (512 - pn0)
                    for j in range(max(0, npair - 2), npair):
                        emit_av(j)

                # evacuate this head's po/prs so the banks free up
                praw_t = normpool.tile([P, PANEL], F32, tag=f"praw{hh}")
                rs_t = normpool.tile([P, PANEL], F32, tag=f"rst{hh}")
                sca.copy(praw_t, po)
                vec.tensor_copy(rs_t, prs)
                nrm.append((praw_t, rs_t))
                if sweep == 1 and p == NP - 1:
                    dm = ps.tile([P, PANEL], F32, tag=f"V{hh}", name="dm")
                    mm(dm, mext[:, 0:128], praw_t, start=True, stop=True)
                    mm(dm, mext[:, 0:128], rs_t, start=True, stop=True)
                yield 100
            last = sweep == 1 and p == NP - 1
            for hh in range(2):
                h = 2 * sweep + hh
                praw_t, rs_t = nrm[hh]
                rinv = normpool.tile([P, PANEL], F32, tag=f"rinv{hh}")
                vec.reciprocal_approx_fast(rinv, rs_t)
                if last:
                    dm2 = ps.tile([P, PANEL], F32, tag=f"V{2 + hh}", name="dm2")
                    mm(dm2, mext[:, 0:128], rinv, start=True, stop=True)
                vec.tensor_mul(outT[h][:, sl], praw_t, rinv)
                if last:
                    dm3 = ps.tile([P, PANEL], F32, tag=f"V{hh}", name="dm3")
                    mm(dm3, onesb, outT[h][:, sl], start=True, stop=True)
            yield 100

    # ---------------- out-proj generator ----------------
    def gen_proj():
        for p in range(NP):
            sl = slice(PANEL * p, PANEL * (p + 1))
            for ob2 in range(KB // 2):
                o_t = pout.tile([P, 2, PANEL], BF16, tag="pout")
                for c in range(2):
                    ob = 2 * ob2 + c
                    bank = ("A2", "A3", "V2", "V3")[ob % 4]
                    pj = ps.tile([P, PANEL], F32, tag=bank, name="pj")
                    for hx in range(HPG):
                        mm(
                            pj,
                            wp_sb[0][:, hx, P * ob : P * (ob + 1)],
                            outT[hx][:, sl],
                            start=(hx == 0),
                            stop=(hx == HPG - 1),
                        )
                    if c == 0:
                        sca.copy(o_t[:, 0], pj)
                    else:
                        vec.tensor_copy(o_t[:, 1], pj)
                    yield 2048
                nc.gpsimd.dma_start(projT3[:, 2 * ob2 : 2 * ob2 + 2, sl], o_t)

    def drive(primary, secondary, ratio):
        """Interleave by emitted-PE-cost: keep secondary's emitted cost at
        ~ratio x primary's, so both generators drain together."""
        pc, sc_acc = 1.0, 0.0
        for c in primary:
            pc += c
            while secondary is not None and sc_acc < pc * ratio:
                nx = next(secondary, None)
                if nx is None:
                    secondary = None
                else:
                    sc_acc += nx
        while secondary is not None:
            if next(secondary, None) is None:
                secondary = None

    # A0 alone (owns all banks)
    for _ in gen_A(0):
        pass
    # B0 interleaved with A1; prime A1's setup DMAs + first matmul chunk
    # so the PE has ready work across the A0->B0 transition
    a1 = gen_A(1)
    next(a1)
    next(a1)
    drive(gen_B(0), a1, 2.2)
    # B1 interleaved with proj(panels 0..2); panel p's proj only becomes
    # emittable after its norm, which drive() handles via emission order:
    # proj is the secondary and trails B1 by construction of gen order.
    drive(gen_B(1), gen_proj(), 2.2)

    if DEBUG:
        for h in range(HPG):
            nc.sync.dma_start(t[f"dbg_o{h}"], outT[h])


def build_nc():
    key = (DEBUG,)
    if key in _NC_CACHE:
        return _NC_CACHE[key]
    nc = bacc.Bacc("TRN2", target_bir_lowering=False, debug=False)
    _bt = nc.alloc_sbuf_tensor(f"const-float32-{EBIAS}", [128, 1], F32)
    nc.gpsimd.memset(_bt.ap(), EBIAS)
    nc.const_aps.aps[(F32, EBIAS)] = _bt.ap()
    nc.all_engine_barrier()
    t = {}
    t["xT"] = nc.dram_tensor("xT", [C, N], BF16, kind="ExternalInput").ap()
    t["wq"] = nc.dram_tensor("wq", [C, 512], BF16, kind="ExternalInput").ap()
    t["wk"] = nc.dram_tensor("wk", [C, 512], BF16, kind="ExternalInput").ap()
    t["wv"] = nc.dram_tensor("wv", [C, 512], BF16, kind="ExternalInput").ap()
    t["wp"] = nc.dram_tensor("wp", [512, N], BF16, kind="ExternalInput").ap()
    t["cosT"] = nc.dram_tensor("cosT", [P, N], BF16, kind="ExternalInput").ap()
    t["sinT"] = nc.dram_tensor("sinT", [P, N], BF16, kind="ExternalInput").ap()
    t["ones8"] = nc.dram_tensor("ones8", [P, 2, P], F8, kind="ExternalInput").ap()
    t["onesb"] = nc.dram_tensor("onesb", [P, P], BF16, kind="ExternalInput").ap()
    t["mext"] = nc.dram_tensor("mext", [P, 2 * P], F32, kind="ExternalInput").ap()
    t["projT"] = nc.dram_tensor("projT", [N, N], BF16, kind="ExternalOutput").ap()
    if DEBUG:
        for h in range(HPG):
            t[f"dbg_o{h}"] = nc.dram_tensor(
                f"dbg_o{h}", [P, N], BF16, kind="ExternalOutput"
            ).ap()
    with tile.TileContext(nc) as tc, ExitStack() as ctx:
        _emit(ctx, tc, t)
    nc.compile()
    _NC_CACHE[key] = nc
    return nc


def make_in_maps(x, position_ids, Wqkv, Wproj):
    x = np.asarray(x, dtype=np.float32)
    pos = np.asarray(position_ids, dtype=np.float64)
    Wqkv = np.asarray(Wqkv, dtype=np.float32)
    Wproj = np.asarray(Wproj, dtype=np.float32)

    inv_freq = 1.0 / (ROPE_BASE ** (np.arange(0, HD, 2, dtype=np.float32) / HD))
    ones8 = np.ones((P, 2, P), dtype=NF8)
    onesb = np.ones((P, P), dtype=NBF16)
    tri = np.where(
        np.arange(P)[None, :] >= np.arange(P)[:, None], 1e4, -1e4
    ).astype(np.float32)
    mext = np.concatenate([np.full((P, P), -1e4, dtype=np.float32), tri], axis=1)

    in_maps = []
    for c in range(8):
        b, g = divmod(c, G)
        freqs = pos[b].astype(np.float32)[:, None] * inv_freq[None, :]
        emb = np.concatenate([freqs, freqs], axis=-1)
        cosT = np.ascontiguousarray(np.cos(emb).T)
        sinT = np.sin(emb)
        sinT = np.ascontiguousarray(sinT.T)
        sinT[:64] = -sinT[:64]
        in_maps.append(
            {
                "xT": np.ascontiguousarray(x[b].T).astype(NBF16),
                "wq": np.ascontiguousarray(Wqkv[:, 512 * g : 512 * (g + 1)]).astype(NBF16),
                "wk": np.ascontiguousarray(
                    Wqkv[:, 2048 + 512 * g : 2048 + 512 * (g + 1)]
                ).astype(NBF16),
                "wv": np.ascontiguousarray(
                    Wqkv[:, 4096 + 512 * g : 4096 + 512 * (g + 1)]
                ).astype(NBF16),
                "wp": np.ascontiguousarray(
                    Wproj[512 * g : 512 * (g + 1), :]
                ).astype(NBF16),
                "cosT": cosT.astype(NBF16),
                "sinT": sinT.astype(NBF16),
                "ones8": ones8,
                "onesb": onesb,
                "mext": mext,
            }
        )
    return in_maps


def kernel(x, position_ids, Wqkv, Wproj, _trace=False, _tmpdir=None):
    nc = build_nc()
    in_maps = make_in_maps(x, position_ids, Wqkv, Wproj)
    res = bass_utils.run_bass_kernel_spmd(
        nc, in_maps, core_ids=list(range(8)), trace=_trace, tmpdir=_tmpdir
    )
    out = np.empty((B, N, C), dtype=np.float32)
    for b in range(B):
        acc = res.results[4 * b]["projT"].astype(np.float32)
        for g in range(1, G):
            acc += res.results[4 * b + g]["projT"].astype(np.float32)
        out[b] = acc.T
    kernel.last_exec_time_ns = res.exec_time_ns
    kernel.last_results = res
    return out


# revision 6
# speedup vs baseline: 1.2167x; 1.0066x over previous
"""Causal attention block (QKV proj + RoPE + causal SDPA + out proj) on 8
Trainium2 NeuronCores — pipelined v3.

Sharding: core c = 4*b + g handles batch b (of 2) and head group g (of 4
heads).  Host sums the 4 bf16 projT partials per batch in fp32.

v3 structure: phase B (attention) is ACT(exp)-bound, so it runs with a
4-PSUM-bank footprint ({A0,A1} score rotation, {V0,V1} po/prs, one head at
a time) while OTHER matmul work runs on the remaining 4 banks
({A2,A3,V2,V3}), interleaved at ~1us emission granularity via generators:

  A0 (QKV+RoPE heads 0-1, v for all heads)  -> [B0  x  A1(QKV heads 2-3)]
  -> [B1 x out-proj(panels 0-2)] -> out-proj(panel 3)

Precision (tol 2e-2, measured ~4e-3): QKV in bf16 (x, Wqkv bf16 -> FWL
weight loads); q/k bf16 after RoPE (scores bf16); e' = exp(s*scale-3) ->
fp8 pair-packed [128,2,512] for panels 1-3 with attn@V + ones-rowsum as
fp8 DoubleRow matmuls (2x PE); panel 0 (few keys -> no noise averaging)
in bf16; the -3 bias keeps exp under fp8 max (240) and cancels in the
softmax ratio; causal diagonal via DVE min-mask (+-1e4) on PSUM before
exp; normalization fp32 with reciprocal_approx_fast off the PE path;
out-proj bf16.  Tail HAM warmth kept by norm-chained dummy matmuls.
"""

import sys

if "/opt/trn_rl_repo" not in sys.path:
    sys.path.insert(0, "/opt/trn_rl_repo")

from contextlib import ExitStack

import ml_dtypes
import numpy as np

import concourse.bass as bass  # noqa: F401
import concourse.tile as tile
from concourse import bacc, bass_utils, mybir

F32 = mybir.dt.float32
F32R = mybir.dt.float32r
BF16 = mybir.dt.bfloat16
F8 = mybir.dt.float8e4
NBF16 = ml_dtypes.bfloat16
NF8 = ml_dtypes.float8_e4m3
EXP = mybir.ActivationFunctionType.Exp
DR = mybir.MatmulPerfMode.DoubleRow

B, N, C = 2, 2048, 2048
H = 16
HD = C // H  # 128
G = 4
HPG = H // G  # 4
P = 128
PANEL = 512
HPAN = 512
NP = N // PANEL  # 4
KB = C // P  # 16
SCALE = float(HD) ** -0.5
EBIAS = -3.0
ROPE_BASE = 10000.0

_NC_CACHE = {}
DEBUG = False


def _emit(ctx, tc, t):
    nc = tc.nc
    mm = nc.tensor.matmul
    sca = nc.scalar
    vec = nc.vector

    const = ctx.enter_context(tc.tile_pool(name="const", bufs=1))
    xpool = ctx.enter_context(tc.tile_pool(name="x", bufs=2))
    wpool = ctx.enter_context(tc.tile_pool(name="w", bufs=1))
    qkpool = ctx.enter_context(tc.tile_pool(name="qk", bufs=1))
    vpool = ctx.enter_context(tc.tile_pool(name="v", bufs=1))
    e8pool = ctx.enter_context(tc.tile_pool(name="e8", bufs=3))
    efpool = ctx.enter_context(tc.tile_pool(name="ef", bufs=3))
    rawpool = ctx.enter_context(tc.tile_pool(name="raw", bufs=2))
    tmppool = ctx.enter_context(tc.tile_pool(name="tmp", bufs=2))
    normpool = ctx.enter_context(tc.tile_pool(name="nrm", bufs=2))
    opool = ctx.enter_context(tc.tile_pool(name="o", bufs=1))
    pout = ctx.enter_context(tc.tile_pool(name="po", bufs=2))
    ps = ctx.enter_context(tc.tile_pool(name="ps", bufs=1, space="PSUM"))

    ones8 = const.tile([P, 2, P], F8)
    onesb = const.tile([P, P], BF16)
    mext = const.tile([P, 2 * P], F32)
    nc.gpsimd.dma_start(ones8, t["ones8"])
    nc.gpsimd.dma_start(onesb, t["onesb"])
    nc.gpsimd.dma_start(mext, t["mext"])
    cosT = const.tile([P, N], BF16)
    sinT = const.tile([P, N], BF16)
    nc.gpsimd.dma_start(cosT, t["cosT"])
    nc.gpsimd.dma_start(sinT, t["sinT"])

    v_sb = vpool.tile([P, KB, PANEL], F8)  # all keys, 4 heads, fp8
    v4 = vpool.tile([P, 4, PANEL], BF16)  # first 512 keys, 4 heads, bf16
    outT = [opool.tile([P, N], BF16, name=f"outT{h}") for h in range(HPG)]

    xT3 = t["xT"].rearrange("(kb q) n -> q kb n", q=P)
    wv3 = t["wv"].rearrange("(kb p) f -> p kb f", p=P)
    wp3 = t["wp"].rearrange("(h p) o -> p h o", p=P)
    projT3 = t["projT"].rearrange("(ob q) n -> q ob n", q=P)

    qk = {}
    for s in range(2):
        for hh in range(2):
            qk["q", hh, s] = qkpool.tile(
                [P, N], BF16, tag=f"q{hh}s{s}", name=f"q{hh}s{s}"
            )
            qk["k", hh, s] = qkpool.tile(
                [P, N], BF16, tag=f"k{hh}s{s}", name=f"k{hh}s{s}"
            )

    wp_sb = [None]

    # ---------------- phase A generator (QKV + RoPE) ----------------
    def gen_A(sweep):
        # bank sets: sweep 0 owns everything; sweep 1 (interleaved with
        # B0) uses only {A2, A3, V2, V3}
        if sweep == 0:
            qtag = ["A0", "A1"]
            ktag = ["A2", "A3"]
        else:
            qtag = ["A2", "A3"]
            ktag = ["V2", "V3"]

        xt0 = xpool.tile([P, KB, HPAN], BF16, tag="x")
        nc.sync.dma_start(xt0[:, 0:2, :], xT3[:, 0:2, 0:HPAN])
        w_sb = {}
        wsrc = {}
        for wname in ("wq", "wk"):
            w_sb[wname] = wpool.tile([P, KB, 256], BF16, tag=wname, name=wname)
            wsrc[wname] = t[wname].rearrange("(kb p) f -> p kb f", p=P)[
                :, :, 256 * sweep : 256 * sweep + 256
            ]
            nc.sync.dma_start(w_sb[wname][:, 0:2, :], wsrc[wname][:, 0:2, :])
        nc.sync.dma_start(xt0[:, 2:8, :], xT3[:, 2:8, 0:HPAN])
        for wname in ("wq", "wk"):
            nc.sync.dma_start(w_sb[wname][:, 2:8, :], wsrc[wname][:, 2:8, :])
        nc.sync.dma_start(xt0[:, 8:16, :], xT3[:, 8:16, 0:HPAN])
        if sweep == 0:
            wv_sb = wpool.tile([P, KB, PANEL], BF16, tag="wv", name="wv")
            for ch in range(8):
                nc.gpsimd.dma_start(
                    wv_sb[:, 2 * ch : 2 * ch + 2, :],
                    wv3[:, 2 * ch : 2 * ch + 2, :],
                )
        else:
            # prefetch wp for the proj phase (reuses the wv slot)
            wp_sb[0] = wpool.tile([P, HPG, N], BF16, tag="wv", name="wp")
            for i in range(HPG):
                nc.gpsimd.dma_start(wp_sb[0][:, i : i + 1, :], wp3[:, i : i + 1, :])
        for wname in ("wq", "wk"):
            nc.sync.dma_start(w_sb[wname][:, 8:16, :], wsrc[wname][:, 8:16, :])
        yield 0

        for hp in range(NP):
            sl = slice(HPAN * hp, HPAN * (hp + 1))
            pq = [
                ps.tile([P, HPAN], F32, tag=qtag[i], name=f"pq{i}")
                for i in range(2)
            ]
            pk = [
                ps.tile([P, HPAN], F32, tag=ktag[i], name=f"pk{i}")
                for i in range(2)
            ]
            if sweep == 0:
                pv = [
                    ps.tile([P, PANEL], F32, tag=f"V{i}", name=f"pv{i}")
                    for i in range(4)
                ]
            if hp == 0:
                xt = xt0
            else:
                xt = xpool.tile([P, KB, HPAN], BF16, tag="x")
                for half in range(2):
                    nc.sync.dma_start(
                        xt[:, 8 * half : 8 * half + 8, :],
                        xT3[:, 8 * half : 8 * half + 8, sl],
                    )
            for kb4 in range(4):
                for kbl in range(4):
                    kb = 4 * kb4 + kbl
                    st, sp = kb == 0, kb == KB - 1
                    x_k = xt[:, kb]
                    mm(pq[0], w_sb["wq"][:, kb, 0:128], x_k, start=st, stop=sp)
                    mm(pq[1], w_sb["wq"][:, kb, 128:256], x_k, start=st, stop=sp)
                    mm(pk[0], w_sb["wk"][:, kb, 0:128], x_k, start=st, stop=sp)
                    mm(pk[1], w_sb["wk"][:, kb, 128:256], x_k, start=st, stop=sp)
                    if sweep == 0:
                        for tb in range(4):
                            mm(
                                pv[tb],
                                x_k[:, P * tb : P * (tb + 1)],
                                wv_sb[:, kb],
                                start=st,
                                stop=sp,
                            )
                yield 16384 if sweep == 0 else 8192
            for psrc, dst in (
                (pq[0], qk["q", 0, sweep]),
                (pq[1], qk["q", 1, sweep]),
                (pk[0], qk["k", 0, sweep]),
                (pk[1], qk["k", 1, sweep]),
            ):
                rawsw = rawpool.tile([P, HPAN], BF16, tag="rawsw")
                sca.copy(rawsw[0:64], psrc[64:128])
                vec.tensor_copy(rawsw[64:128], psrc[0:64])
                t1 = tmppool.tile([P, HPAN], BF16, tag="rope1")
                t2 = tmppool.tile([P, HPAN], BF16, tag="rope2")
                vec.tensor_mul(t2, psrc, cosT[:, sl])
                vec.tensor_mul(t1, rawsw, sinT[:, sl])
                vec.tensor_add(dst[:, sl], t2, t1)
            if sweep == 0:
                for tb in range(4):
                    tbg = 4 * hp + tb
                    sca.copy(v_sb[:, tbg, :], pv[tb])
                    if tbg < 4:
                        sca.copy(v4[:, tbg, :], pv[tb])
            yield 200

    # ---------------- phase B generator (causal SDPA) ----------------
    def gen_B(sweep):
        for p in range(NP):
            sl = slice(PANEL * p, PANEL * (p + 1))
            nrm = []
            for hh in range(2):
                h = 2 * sweep + hh
                po = ps.tile([P, PANEL], F32, tag="V0", name="po")
                prs = ps.tile([P, PANEL], F32, tag="V1", name="prs")

                def emit_sc(jb, n0, mask_lo):
                    sc = ps.tile([P, PANEL], F32, tag=f"A{jb % 2}", name="sc")
                    mm(
                        sc[:, n0:],
                        qk["k", hh, sweep][:, P * jb : P * (jb + 1)],
                        qk["q", hh, sweep][:, PANEL * p + n0 : PANEL * (p + 1)],
                    )
                    td = jb - 4 * p
                    if td >= 0:
                        lo = mask_lo if mask_lo is not None else P * td
                        width = P * (td + 1) - lo
                        vec.tensor_tensor(
                            sc[:, lo : P * (td + 1)],
                            sc[:, lo : P * (td + 1)],
                            mext[:, 2 * P - width :],
                            mybir.AluOpType.min,
                        )
                    return sc

                if p == 0:
                    e_tiles = []

                    def emit_av0(jb):
                        e_t, n0 = e_tiles[jb]
                        st, sp = jb == 0, jb == 3
                        mm(
                            po[:, n0:],
                            v4[:, jb, P * h : P * (h + 1)],
                            e_t[:, n0:],
                            start=st,
                            stop=sp,
                        )
                        mm(prs[:, n0:], onesb, e_t[:, n0:], start=st, stop=sp)

                    for jb in range(4):
                        n0 = P * jb
                        if jb >= 2:
                            emit_av0(jb - 2)
                        sc = emit_sc(jb, n0, None)
                        e_t = efpool.tile([P, PANEL], BF16, tag="ef")
                        sca.activation(
                            e_t[:, n0:], sc[:, n0:], EXP, scale=SCALE, bias=EBIAS
                        )
                        e_tiles.append((e_t, n0))
                        yield (512 - n0) * 5 // 2
                    for jb in range(2, 4):
                        emit_av0(jb)
                else:
                    npair = 2 * p + 2
                    e_tiles = []

                    def emit_av(j):
                        e_t, pn0 = e_tiles[j]
                        st, sp = j == 0, j == npair - 1
                        mm(
                            po[:, pn0:],
                            v_sb[:, 2 * j : 2 * j + 2, P * h : P * (h + 1)],
                            e_t[:, :, pn0:],
                            start=st,
                            stop=sp,
                            perf_mode=DR,
                        )
                        mm(
                            prs[:, pn0:],
                            ones8,
                            e_t[:, :, pn0:],
                            start=st,
                            stop=sp,
                            perf_mode=DR,
                        )

                    for j in range(npair):
                        pn0 = 256 if j == 2 * p + 1 else 0
                        if j >= 2:
                            emit_av(j - 2)
                        e_t = e8pool.tile([P, 2, PANEL], F8, tag="e8")
                        for cc in range(2):
                            jb = 2 * j + cc
                            sc = emit_sc(jb, pn0, pn0 if cc == 1 else None)
                            sca.activation(
                                e_t[:, cc, pn0:],
                                sc[:, pn0:],
                                EXP,
                                scale=SCALE,
                                bias=EBIAS,
                            )
                        e_tiles.append((e_t, pn0))
                        yield 3 * (512 - pn0)
                    for j in range(max(0, npair - 2), npair):
                        emit_av(j)

                # evacuate this head's po/prs so the banks free up
                praw_t = normpool.tile([P, PANEL], F32, tag=f"praw{hh}")
                rs_t = normpool.tile([P, PANEL], F32, tag=f"rst{hh}")
                sca.copy(praw_t, po)
                vec.tensor_copy(rs_t, prs)
                nrm.append((praw_t, rs_t))
                if sweep == 1 and p == NP - 1:
                    dm = ps.tile([P, PANEL], F32, tag=f"V{hh}", name="dm")
                    mm(dm, mext[:, 0:128], praw_t, start=True, stop=True)
                    mm(dm, mext[:, 0:128], rs_t, start=True, stop=True)
                yield 100
            last = sweep == 1 and p == NP - 1
            for hh in range(2):
                h = 2 * sweep + hh
                praw_t, rs_t = nrm[hh]
                rinv = normpool.tile([P, PANEL], F32, tag=f"rinv{hh}")
                vec.reciprocal_approx_fast(rinv, rs_t)
                if last:
                    dm2 = ps.tile([P, PANEL], F32, tag=f"V{2 + hh}", name="dm2")
                    mm(dm2, mext[:, 0:128], rinv, start=True, stop=True)
                vec.tensor_mul(outT[h][:, sl], praw_t, rinv)
                if last:
                    dm3 = ps.tile([P, PANEL], F32, tag=f"V{hh}", name="dm3")
                    mm(dm3, onesb, outT[h][:, sl], start=True, stop=True)
            yield 100

    # ---------------- out-proj generator ----------------
    def gen_proj():
        for p in range(NP):
            sl = slice(PANEL * p, PANEL * (p + 1))
            for ob2 in range(KB // 2):
                o_t = pout.tile([P, 2, PANEL], BF16, tag="pout")
                for c in range(2):
                    ob = 2 * ob2 + c
                    bank = ("A2", "A3", "V2", "V3")[ob % 4]
                    pj = ps.tile([P, PANEL], F32, tag=bank, name="pj")
                    for hx in range(HPG):
                        mm(
                            pj,
                            wp_sb[0][:, hx, P * ob : P * (ob + 1)],
                            outT[hx][:, sl],
                            start=(hx == 0),
                            stop=(hx == HPG - 1),
                        )
                    if c == 0:
                        sca.copy(o_t[:, 0], pj)
                    else:
                        vec.tensor_copy(o_t[:, 1], pj)
                    yield 2048
                nc.gpsimd.dma_start(projT3[:, 2 * ob2 : 2 * ob2 + 2, sl], o_t)

    def drive(primary, secondary, ratio):
        """Interleave by emitted-PE-cost: keep secondary's emitted cost at
        ~ratio x primary's, so both generators drain together."""
        pc, sc_acc = 1.0, 0.0
        for c in primary:
            pc += c
            while secondary is not None and sc_acc < pc * ratio:
                nx = next(secondary, None)
                if nx is None:
                    secondary = None
                else:
                    sc_acc += nx
        while secondary is not None:
            if next(secondary, None) is None:
                secondary = None

    # A0 alone (owns all banks)
    for _ in gen_A(0):
        pass
    # B0 interleaved with A1; prime A1's setup DMAs + first matmul chunk
    # so the PE has ready work across the A0->B0 transition
    a1 = gen_A(1)
    next(a1)
    next(a1)
    drive(gen_B(0), a1, 2.2)
    # B1 interleaved with proj(panels 0..2); panel p's proj only becomes
    # emittable after its norm, which drive() handles via emission order:
    # proj is the secondary and trails B1 by construction of gen order.
    drive(gen_B(1), gen_proj(), 2.2)

    if DEBUG:
        for h in range(HPG):
            nc.sync.dma_start(t[f"dbg_o{h}"], outT[h])


def build_nc():
    key = (DEBUG,)
    if key in _NC_CACHE:
        return _NC_CACHE[key]
    nc = bacc.Bacc("TRN2", target_bir_lowering=False, debug=False)
    _bt = nc.alloc_sbuf_tensor(f"const-float32-{EBIAS}", [128, 1], F32)
    nc.gpsimd.memset(_bt.ap(), EBIAS)
    nc.const_aps.aps[(F32, EBIAS)] = _bt.ap()
    nc.all_engine_barrier()
    t = {}
    t["xT"] = nc.dram_tensor("xT", [C, N], BF16, kind="ExternalInput").ap()
    t["wq"] = nc.dram_tensor("wq", [C, 512], BF16, kind="ExternalInput").ap()
    t["wk"] = nc.dram_tensor("wk", [C, 512], BF16, kind="ExternalInput").ap()
    t["wv"] = nc.dram_tensor("wv", [C, 512], BF16, kind="ExternalInput").ap()
    t["wp"] = nc.dram_tensor("wp", [512, N], BF16, kind="ExternalInput").ap()
    t["cosT"] = nc.dram_tensor("cosT", [P, N], BF16, kind="ExternalInput").ap()
    t["sinT"] = nc.dram_tensor("sinT", [P, N], BF16, kind="ExternalInput").ap()
    t["ones8"] = nc.dram_tensor("ones8", [P, 2, P], F8, kind="ExternalInput").ap()
    t["onesb"] = nc.dram_tensor("onesb", [P, P], BF16, kind="ExternalInput").ap()
    t["mext"] = nc.dram_tensor("mext", [P, 2 * P], F32, kind="ExternalInput").ap()
    t["projT"] = nc.dram_tensor("projT", [N, N], BF16, kind="ExternalOutput").ap()
    if DEBUG:
        for h in range(HPG):
            t[f"dbg_o{h}"] = nc.dram_tensor(
                f"dbg_o{h}", [P, N], BF16, kind="ExternalOutput"
            ).ap()
    with tile.TileContext(nc) as tc, ExitStack() as ctx:
        _emit(ctx, tc, t)
    nc.compile()
    _NC_CACHE[key] = nc
    return nc


def make_in_maps(x, position_ids, Wqkv, Wproj):
    x = np.asarray(x, dtype=np.float32)
    pos = np.asarray(position_ids, dtype=np.float64)
    Wqkv = np.asarray(Wqkv, dtype=np.float32)
    Wproj = np.asarray(Wproj, dtype=np.float32)

    inv_freq = 1.0 / (ROPE_BASE ** (np.arange(0, HD, 2, dtype=np.float32) / HD))
    ones8 = np.ones((P, 2, P), dtype=NF8)
    onesb = np.ones((P, P), dtype=NBF16)
    tri = np.where(
        np.arange(P)[None, :] >= np.arange(P)[:, None], 1e4, -1e4
    ).astype(np.float32)
    mext = np.concatenate([np.full((P, P), -1e4, dtype=np.float32), tri], axis=1)

    in_maps = []
    for c in range(8):
        b, g = divmod(c, G)
        freqs = pos[b].astype(np.float32)[:, None] * inv_freq[None, :]
        emb = np.concatenate([freqs, freqs], axis=-1)
        cosT = np.ascontiguousarray(np.cos(emb).T)
        sinT = np.sin(emb)
        sinT = np.ascontiguousarray(sinT.T)
        sinT[:64] = -sinT[:64]
        in_maps.append(
            {
                "xT": np.ascontiguousarray(x[b].T).astype(NBF16),
                "wq": np.ascontiguousarray(Wqkv[:, 512 * g : 512 * (g + 1)]).astype(NBF16),
                "wk": np.ascontiguousarray(
                    Wqkv[:, 2048 + 512 * g : 2048 + 512 * (g + 1)]
                ).astype(NBF16),
                "wv": np.ascontiguousarray(
                    Wqkv[:, 4096 + 512 * g : 4096 + 512 * (g + 1)]
                ).astype(NBF16),
                "wp": np.ascontiguousarray(
                    Wproj[512 * g : 512 * (g + 1), :]
                ).astype(NBF16),
                "cosT": cosT.astype(NBF16),
                "sinT": sinT.astype(NBF16),
                "ones8": ones8,
                "onesb": onesb,
                "mext": mext,
            }
        )
    return in_maps


def kernel(x, position_ids, Wqkv, Wproj, _trace=False, _tmpdir=None):
    nc = build_nc()
    in_maps = make_in_maps(x, position_ids, Wqkv, Wproj)
    res = bass_utils.run_bass_kernel_spmd(
        nc, in_maps, core_ids=list(range(8)), trace=_trace, tmpdir=_tmpdir
    )
    out = np.empty((B, N, C), dtype=np.float32)
    for b in range(B):
        acc = res.results[4 * b]["projT"].astype(np.float32)
        for g in range(1, G):
            acc += res.results[4 * b + g]["projT"].astype(np.float32)
        out[b] = acc.T
    kernel.last_exec_time_ns = res.exec_time_ns
    kernel.last_results = res
    return out
